# revision 2
# baseline (speedup 1.0000x reference)
# Trainium2 Bass kernel for nn_CrossAttention_noise (B=4, T1=T2=1024, D=1024,
# H=16, DK=64, KW=13, FF=4096), SPMD over 8 NeuronCores.
#
# Sharding: core i handles batch b=i//2 and query-token half t0=(i%2)*512.
# All heavy matmuls run in fp8e4 with DoubleRow perf mode (2 K-tiles per
# pass):
#   - grouped convs: 2 heads per matmul via block-diagonal weights; the two
#     K-tiles are the shift-0/shift-1 planes of the transposed input (the
#     shift-1 plane is a SBUF->SBUF DMA copy), so one DR matmul covers 2 taps
#     x 128 channels; 7 matmuls cover the padded 14 taps.
#   - scores: lhsT/rhs use a stride-0 "pair broadcast" so the DR pass
#     computes 2*k^T q; the exp compensates with scale 1/16.
#   - PV (token-major out), fc, FFN: K-tile pairs are adjacent 128-ch blocks.
# The key-padding mask is folded into v65 (value rows and the ones-column
# multiplied by 0/1), so exp needs no per-chunk bias and one Activation op
# covers two score chunks (a 2-bank PSUM tile).
import numpy as np
import ml_dtypes
from contextlib import ExitStack

import concourse.bass as bass
import concourse.mybir as mybir
import concourse.tile as tile
from concourse import bacc
from concourse.bass_utils import run_bass_kernel_spmd
from concourse.masks import make_identity

BF16 = mybir.dt.bfloat16
F32 = mybir.dt.float32
FP8 = mybir.dt.float8e4
DR = mybir.MatmulPerfMode.DoubleRow
AF = mybir.ActivationFunctionType
ALU = mybir.AluOpType
AX = mybir.AxisListType

B, T, D, H, DK, KW, FF = 4, 1024, 1024, 16, 64, 13, 4096
TQ = 512           # query tokens per core
P = 128
NHW = 768          # noisy halo window rows (zero-padded on host)
NTW = 528          # nt2 plane width (524 used + 4 pad)
CTW = 1040         # ct2 plane width (1036 used + 4 pad)
EPS1, EPS2 = 1e-5, 1e-6
WS = 32.0          # host-side weight scale (power of two)
WSI = 1.0 / WS


def pair_b(ap2d):
    """[p, N] -> [p, 2, N] stride-0 plane broadcast (for double-q scores)."""
    p, n = ap2d.shape
    return ap2d.unsqueeze(1).broadcast_to((p, 2, n))


def build_nc():
    nc = bacc.Bacc("TRN2", target_bir_lowering=False, debug=False,
                   num_devices=8)
    dt = {}

    def din(name, shape, dtype):
        dt[name] = nc.dram_tensor(name, list(shape), dtype,
                                  kind="ExternalInput").ap()

    din("noisyH", (NHW, D), F32)       # rows [t0-128, t0+640), zero padded
    din("clean", (T, D), F32)
    din("mod", (10, D), F32)           # sh_m,1+sc_m,g_m,sh_f,1+sc_f,g_f,
    #                                     shL,(1+sc)L,shR,(1+sc)R (edge-masked)
    din("clng", (D,), F32)
    din("clnb", (D,), F32)
    din("wq", (P, 8, 7, 2, P), FP8)    # [cin][pair][tap-pair j][plane][cout]
    din("wk", (P, 8, 7, 2, P), FP8)
    din("wv", (P, 8, 7, 2, P), FP8)
    din("bq", (D,), F32)
    din("bk", (D,), F32)
    din("bv", (D,), F32)
    din("mk", (P, 8, 2, DK), FP8)      # key mask replicated (part,chunk,h2,dk)
    din("fcw", (P, 8, 4, 2, P), FP8)   # [ic][m][j][plane][oc]
    din("fcb", (D,), F32)
    din("w1t", (P, 32, 8, P), BF16)    # [ic][m][k][oc]
    din("fb1", (FF,), F32)
    din("w2t", (P, 8, 32, P), BF16)
    din("fb2", (D,), F32)
    out_ap = nc.dram_tensor("out", [TQ, D], F32, kind="ExternalOutput").ap()

    with tile.TileContext(nc) as tc:
        _emit(tc, dt, out_ap)
    nc.compile()
    return nc


def _emit(tc, dt, out_ap):
    nc = tc.nc
    with ExitStack() as ctx:
        const = ctx.enter_context(tc.tile_pool(name="const", bufs=1))
        small = ctx.enter_context(tc.tile_pool(name="small", bufs=4))
        lnio = ctx.enter_context(tc.tile_pool(name="lnio", bufs=4))
        big = ctx.enter_context(tc.tile_pool(name="bigsb", bufs=1))
        wpool = ctx.enter_context(tc.tile_pool(name="wstream", bufs=3))
        pcv = ctx.enter_context(tc.tile_pool(name="pcv", bufs=1, space="PSUM"))
        psc = ctx.enter_context(tc.tile_pool(name="psc", bufs=2, space="PSUM"))
        ptp = ctx.enter_context(tc.tile_pool(name="ptp", bufs=2, space="PSUM"))
        ppv = ctx.enter_context(tc.tile_pool(name="ppv", bufs=1, space="PSUM"))

        ident = const.tile([P, P], BF16)
        make_identity(nc, ident)
        e_1 = const.tile([P, 1], F32)
        nc.vector.memset(e_1, EPS1)
        e_12 = const.tile([P, 1], F32)
        nc.vector.memset(e_12, EPS1 * EPS2)
        e_2 = const.tile([P, 1], F32)
        nc.vector.memset(e_2, EPS2)
        EPS_T = {"1": e_1, "12": e_12, "2": e_2}

        def chanvec(name, w=8):
            t = const.tile([P, w], F32, tag=f"cv_{name}")
            nc.sync.dma_start(t, dt[name].rearrange("(m p) -> p m", p=P))
            return t

        bq_s, bk_s, bv_s = chanvec("bq"), chanvec("bk"), chanvec("bv")
        fcb_s, fb2_s = chanvec("fcb"), chanvec("fb2")
        clng_s, clnb_s = chanvec("clng"), chanvec("clnb")
        fb1_s = chanvec("fb1", 32)
        mod_s = const.tile([P, 10, 8], F32)
        for s in range(10):
            nc.sync.dma_start(mod_s[:, s, :],
                              dt["mod"][s].rearrange("(m p) -> p m", p=P))
        sh_msa, sc_msa, g_msa = mod_s[:, 0, :], mod_s[:, 1, :], mod_s[:, 2, :]
        sh_mlp, sc_mlp, g_mlp = mod_s[:, 3, :], mod_s[:, 4, :], mod_s[:, 5, :]
        shL, scL = mod_s[:, 6, :], mod_s[:, 7, :]
        shR, scR = mod_s[:, 8, :], mod_s[:, 9, :]
        mk_s = const.tile([P, 8, 2, DK], FP8)
        nc.sync.dma_start(mk_s, dt["mk"])

        def stats(x, eng_sq):
            """(mu, var) [p,1] f32 for rows of x [p, D]."""
            s = small.tile([P, 1], F32, tag="st_s")
            nc.vector.reduce_sum(s, x, axis=AX.X)
            sq = small.tile([P, 1], F32, tag="st_sq")
            scr = small.tile([P, D], BF16, tag="st_scr", bufs=2)
            nc.scalar.activation(scr, x, AF.Square, accum_out=sq)
            mu = small.tile([P, 1], F32, tag="st_mu")
            nc.vector.tensor_scalar_mul(mu, s, 1.0 / D)
            msq = small.tile([P, 1], F32, tag="st_msq")
            nc.vector.tensor_tensor(msq, mu, mu, ALU.mult)
            var = small.tile([P, 1], F32, tag="st_var")
            nc.vector.tensor_scalar(var, sq, 1.0 / D, msq,
                                    ALU.mult, ALU.subtract)
            return mu, var

        def rs_beta(mu, var, eps_key, scale, tag):
            """(r, beta) with r = 1/sqrt(var*scale+eps), beta = -mu*r."""
            st = small.tile([P, 1], F32, tag=tag + "_s")
            nc.scalar.activation(st, var, AF.Sqrt, bias=EPS_T[eps_key],
                                 scale=scale)
            rt = small.tile([P, 1], F32, tag=tag + "_r")
            nc.vector.reciprocal(rt, st)
            bt = small.tile([P, 1], F32, tag=tag + "_b")
            nc.vector.tensor_tensor(bt, mu, rt, ALU.mult)
            nc.vector.tensor_scalar_mul(bt, bt, -1.0)
            return rt, bt

        xres = big.tile([P, 4, D], F32)        # y1 rows [t0, t0+512); then x
        abc_cm = tc.tile_pool(name="abc", bufs=1)
        abc = abc_cm.__enter__()
        nt2 = [abc.tile([P, 2, NTW], FP8, name=f"nt2_{m}") for m in range(8)]
        ct2 = [abc.tile([P, 2, CTW], FP8, name=f"ct2_{m}") for m in range(8)]
        qT = [abc.tile([P, TQ], FP8, name=f"qT_{m}") for m in range(8)]
        attnT = big.tile([P, 8, TQ], FP8)
        attnQ = big.tile([P, 4, H, DK], BF16)

        wqs = abc.tile([P, 8, 7, 2, P], FP8, name="wqs")
        wks = abc.tile([P, 8, 7, 2, P], FP8, name="wks")
        wvs = abc.tile([P, 8, 7, 2, P], FP8, name="wvs")

        # one-time pad zeroing (tiles are persistent)
        for m in range(8):
            nc.gpsimd.memset(nt2[m][:, 0, 524:528], 0.0)
            nc.gpsimd.memset(nt2[m][:, 1, 523:528], 0.0)
            nc.gpsimd.memset(ct2[m][:, 0, 0:6], 0.0)
            nc.gpsimd.memset(ct2[m][:, 0, 1030:1040], 0.0)
            nc.gpsimd.memset(ct2[m][:, 1, 1029:1040], 0.0)

        # ---- Phase A: noisy stats -> y1 (xres) + y2 -> nt2 + Q convs --------
        with tc.tile_pool(name="lnA", bufs=1) as lnA:
            y2 = [lnA.tile([P, D], BF16, name=f"y2_{r}") for r in range(6)]
            for r in range(6):
                xt = lnio.tile([P, D], F32, tag="ln_in", bufs=2)
                nc.sync.dma_start(xt, dt["noisyH"][r * P:(r + 1) * P, :])
                mu, var = stats(xt, "act" if r % 2 else "dve")
                # fused LN2(LN1(x)) for gamma=1,beta=0:
                #   y2 = (x-mu)/sqrt(var*(1+eps2) + eps1*eps2)
                r2, b2 = rs_beta(mu, var, "12", 1.0 + EPS2, "r2")
                nc.gpsimd.tensor_scalar(y2[r], xt, r2, b2, ALU.mult,
                                        ALU.add)
                if 1 <= r <= 4:
                    r1, b1 = rs_beta(mu, var, "1", 1.0, "r1")
                    nc.gpsimd.tensor_scalar(xres[:, r - 1, :], xt, r1, b1,
                                            ALU.mult, ALU.add)
            for m in range(8):
                bA = ptp.tile([P, 4, P], BF16, tag="tpb")
                bB = ptp.tile([P, 4, P], BF16, tag="tpb")
                for r in range(6):
                    dst = bA[:, r, :] if r < 4 else bB[:, r - 4, :]
                    nc.tensor.transpose(dst, y2[r][:, m * P:(m + 1) * P],
                                        ident)
                mm, ms = sc_msa[:, m:m + 1], sh_msa[:, m:m + 1]
                nc.vector.tensor_scalar(nt2[m][:, 0, 0:6], bA[:, 0, 122:128],
                                        scL[:, m:m + 1], shL[:, m:m + 1],
                                        ALU.mult, ALU.add)
                nc.vector.tensor_scalar(
                    nt2[m][:, 0, 6:390],
                    bA.rearrange("p a b -> p (a b)")[:, 128:512],
                    mm, ms, ALU.mult, ALU.add)
                nc.vector.tensor_scalar(nt2[m][:, 0, 390:518],
                                        bB[:, 0, :], mm, ms,
                                        ALU.mult, ALU.add)
                nc.vector.tensor_scalar(nt2[m][:, 0, 518:524], bB[:, 1, 0:6],
                                        scR[:, m:m + 1], shR[:, m:m + 1],
                                        ALU.mult, ALU.add)
                nc.sync.dma_start(nt2[m][:, 1, 0:523], nt2[m][:, 0, 1:524])
        nc.sync.dma_start(wqs, dt["wq"])
        for hp in range(8):
            ps = pcv.tile([P, TQ], F32, tag="conv")
            for j in range(7):
                nc.tensor.matmul(ps, wqs[:, hp, j],
                                 nt2[hp][:, :, 2 * j:2 * j + TQ],
                                 start=(j == 0), stop=(j == 6), perf_mode=DR)
            nc.vector.tensor_scalar(qT[hp], ps, WSI, bq_s[:, hp:hp + 1],
                                    ALU.mult, ALU.add)

        # ---- Phase B: clean stats -> y -> ct2 -------------------------------
        with tc.tile_pool(name="lnB", bufs=1) as lnB:
            yc = [lnB.tile([P, D], BF16, name=f"yc_{r}") for r in range(8)]
            for r in range(8):
                xt = lnio.tile([P, D], F32, tag="ln_in", bufs=2)
                nc.sync.dma_start(xt, dt["clean"][r * P:(r + 1) * P, :])
                mu, var = stats(xt, "act" if r % 2 else "dve")
                r1, b1 = rs_beta(mu, var, "1", 1.0, "rc")
                nc.gpsimd.tensor_scalar(yc[r], xt, r1, b1, ALU.mult,
                                        ALU.add)
            for m in range(8):
                gg, bb = clng_s[:, m:m + 1], clnb_s[:, m:m + 1]
                for half in range(2):
                    bk4 = ptp.tile([P, 4, P], BF16, tag="tpb")
                    for r4 in range(4):
                        nc.tensor.transpose(
                            bk4[:, r4, :],
                            yc[half * 4 + r4][:, m * P:(m + 1) * P], ident)
                    nc.vector.tensor_scalar(
                        ct2[m][:, 0, 6 + half * 512:518 + half * 512],
                        bk4.rearrange("p a b -> p (a b)"), gg, bb,
                        ALU.mult, ALU.add)
                nc.sync.dma_start(ct2[m][:, 1, 0:1039], ct2[m][:, 0, 1:1040])

        # ---- Phase C: per-pair K/V conv + attention -------------------------
        nc.sync.dma_start(wks, dt["wk"])
        nc.sync.dma_start(wvs, dt["wv"])
        with tc.tile_pool(name="hpool", bufs=2) as hpool:
            for hp in range(8):
                kT = hpool.tile([P, 2, TQ], FP8, tag="kT")
                for c in range(2):
                    ps = pcv.tile([P, TQ], F32, tag="conv")
                    for j in range(7):
                        nc.tensor.matmul(
                            ps, wks[:, hp, j],
                            ct2[hp][:, :, 2 * j + c * TQ:2 * j + (c + 1) * TQ],
                            start=(j == 0), stop=(j == 6), perf_mode=DR)
                    nc.vector.tensor_scalar(kT[:, c, :], ps, WSI,
                                            bk_s[:, hp:hp + 1],
                                            ALU.mult, ALU.add)
                vT = hpool.tile([P, 2, TQ], BF16, tag="vT")
                for c in range(2):
                    ps = pcv.tile([P, TQ], F32, tag="conv")
                    for j in range(7):
                        nc.tensor.matmul(
                            ps, wvs[:, hp, j],
                            ct2[hp][:, :, 2 * j + c * TQ:2 * j + (c + 1) * TQ],
                            start=(j == 0), stop=(j == 6), perf_mode=DR)
                    nc.vector.tensor_scalar(vT[:, c, :], ps, WSI,
                                            bv_s[:, hp:hp + 1],
                                            ALU.mult, ALU.add)
                # v65: token-major masked values + masked ones column
                v65 = hpool.tile([P, 8, 2, 80], FP8, tag="v65")
                nc.vector.tensor_copy(v65[:, :, :, 64], mk_s[:, :, :, 0])
                vTf = vT.rearrange("p a b -> p (a b)")
                for half in range(2):
                    bk4 = ptp.tile([P, 4, P], BF16, tag="tpb")
                    for c4 in range(4):
                        cc = half * 4 + c4
                        nc.tensor.transpose(bk4[:, c4, :],
                                            vTf[:, cc * P:(cc + 1) * P],
                                            ident)
                    nc.vector.tensor_tensor(
                        v65[:, half * 4:half * 4 + 4, :, 0:64],
                        bk4.rearrange("p a (h d) -> p a h d", h=2),
                        mk_s[:, half * 4:half * 4 + 4, :, :], ALU.mult)
                kTf = kT.rearrange("p a b -> p (a b)")
                for hh in range(2):
                    h = 2 * hp + hh
                    rows = slice(hh * DK, (hh + 1) * DK)
                    pT = hpool.tile([P, 8, TQ], FP8, tag="pT")
                    for c2 in range(4):
                        ps = psc.tile([P, 2, TQ], F32, tag="sc")
                        for c in range(2):
                            cc = 2 * c2 + c
                            nc.tensor.matmul(
                                ps[:, c, :],
                                pair_b(kTf[rows, cc * P:(cc + 1) * P]),
                                pair_b(qT[hp][rows, :]),
                                start=True, stop=True, perf_mode=DR)
                        nc.scalar.activation(
                            pT[:, 2 * c2:2 * c2 + 2, :].rearrange(
                                "p a b -> p (a b)"),
                            ps.rearrange("p a b -> p (a b)"), AF.Exp,
                            scale=0.0625)
                    pv = ppv.tile([P, 4, 65], F32, tag="pv")
                    for jq in range(4):
                        for c2 in range(4):
                            nc.tensor.matmul(
                                pv[:, jq, :],
                                pT[:, 2 * c2:2 * c2 + 2, jq * P:(jq + 1) * P],
                                v65[:, 2 * c2:2 * c2 + 2, hh, 0:65],
                                start=(c2 == 0), stop=(c2 == 3), perf_mode=DR)
                    rec = small.tile([P, 4], F32, tag="rec")
                    nc.vector.reciprocal(rec, pv[:, :, 64])
                    for jq in range(4):
                        nc.vector.tensor_scalar_mul(attnQ[:, jq, h, :],
                                                    pv[:, jq, 0:64],
                                                    rec[:, jq:jq + 1])

        abc_cm.__exit__(None, None, None)

        # ---- Phase D: attnT transposes + fc + residual ----------------------
        aqf = attnQ.rearrange("p a h d -> p (a h d)")
        for hp in range(8):
            bk4 = ptp.tile([P, 4, P], BF16, tag="tpb")
            for jq in range(4):
                nc.tensor.transpose(
                    bk4[:, jq, :],
                    aqf[:, jq * 1024 + hp * P:jq * 1024 + (hp + 1) * P],
                    ident)
            nc.vector.tensor_copy(attnT[:, hp, :],
                                  bk4.rearrange("p a b -> p (a b)"))
        ffw = ctx.enter_context(tc.tile_pool(name="ffw", bufs=1))
        fcws = ffw.tile([P, 8, 4, 2, P], FP8, name="fcws")
        nc.sync.dma_start(fcws, dt["fcw"])
        for m2 in range(4):
            ps2 = psc.tile([P, 2, TQ], F32, tag="sc")
            for half in range(2):
                for j in range(4):
                    nc.tensor.matmul(ps2[:, half, :],
                                     fcws[:, 2 * m2 + half, j],
                                     attnT[:, 2 * j:2 * j + 2, :],
                                     start=(j == 0), stop=(j == 3),
                                     perf_mode=DR)
          # per-m evacs below read ps2 planes
            fcgs2 = []
            for half in range(2):
                m = 2 * m2 + half
                ps = ps2[:, half, :]
                fcg = lnio.tile([P, TQ], BF16, tag="fcg", bufs=4)
                gb = small.tile([P, 1], F32, tag="gb")
                nc.vector.tensor_tensor(gb, fcb_s[:, m:m + 1],
                                        g_msa[:, m:m + 1], ALU.mult)
                gsc = small.tile([P, 1], F32, tag="gsc")
                nc.vector.tensor_scalar_mul(gsc, g_msa[:, m:m + 1], WSI)
                nc.scalar.activation(fcg, ps, AF.Identity, bias=gb, scale=gsc)
                fcgs2.append(fcg)
            for half in range(2):
                m = 2 * m2 + half
                fcg = fcgs2[half]
                bk4 = ptp.tile([P, 4, P], BF16, tag="tpb")
                for jq in range(4):
                    nc.tensor.transpose(bk4[:, jq, :],
                                        fcg[:, jq * P:(jq + 1) * P], ident)
                nc.vector.tensor_tensor(xres[:, :, m * P:(m + 1) * P], bk4,
                                        xres[:, :, m * P:(m + 1) * P],
                                        ALU.add)

        # ---- Phase E: LN3 + mlp modulation -> n2T ---------------------------
        n2T = big.tile([P, 8, TQ], BF16)
        with tc.tile_pool(name="lnE", bufs=1) as lnE:
            l3 = [lnE.tile([P, D], BF16, name=f"l3_{s}") for s in range(4)]
            for s in range(4):
                mu, var = stats(xres[:, s, :], "act" if s % 2 else "dve")
                r3, b3 = rs_beta(mu, var, "2", 1.0, "r3")
                nc.gpsimd.tensor_scalar(l3[s], xres[:, s, :], r3, b3,
                                        ALU.mult, ALU.add)
            for m in range(8):
                bk4 = ptp.tile([P, 4, P], BF16, tag="tpb")
                for s in range(4):
                    nc.tensor.transpose(bk4[:, s, :],
                                        l3[s][:, m * P:(m + 1) * P], ident)
                nc.vector.tensor_scalar(n2T[:, m, :],
                                        bk4.rearrange("p a b -> p (a b)"),
                                        sc_mlp[:, m:m + 1],
                                        sh_mlp[:, m:m + 1],
                                        ALU.mult, ALU.add)

        # ---- Phase F: FFN ---------------------------------------------------
        with tc.tile_pool(name="ffp", bufs=1) as ffp:
            ffa = ffp.tile([P, 32, TQ], BF16)
            for m8 in range(4):
                w1c = wpool.tile([P, 8, 8, P], BF16, tag="w1c", bufs=2)
                nc.sync.dma_start(w1c, dt["w1t"][:, m8 * 8:(m8 + 1) * 8])
                for m2 in range(4):
                    ps2 = psc.tile([P, 2, TQ], F32, tag="sc")
                    for half in range(2):
                        m_ = 2 * m2 + half
                        m = m8 * 8 + m_
                        for j in range(8):
                            nc.tensor.matmul(ps2[:, half, :], w1c[:, m_, j],
                                             n2T[:, j, :],
                                             start=(j == 0), stop=(j == 7))
                        nc.scalar.activation(ffa[:, m, :], ps2[:, half, :],
                                             AF.Gelu_apprx_tanh,
                                             bias=fb1_s[:, m:m + 1])
            for m2 in range(4):
                ps2 = psc.tile([P, 2, TQ], F32, tag="sc")
                ffgs2 = []
                for half in range(2):
                    m = 2 * m2 + half
                    w2c = wpool.tile([P, 32, P], BF16, tag="w2c", bufs=3)
                    nc.sync.dma_start(w2c, dt["w2t"][:, m])
                    for j in range(32):
                        nc.tensor.matmul(ps2[:, half, :], w2c[:, j],
                                         ffa[:, j, :],
                                         start=(j == 0), stop=(j == 31))
                    ps = ps2[:, half, :]
                    ffg = lnio.tile([P, TQ], BF16, tag="ffg", bufs=4)
                    gb = small.tile([P, 1], F32, tag="gb2")
                    nc.vector.tensor_tensor(gb, fb2_s[:, m:m + 1],
                                            g_mlp[:, m:m + 1], ALU.mult)
                    nc.scalar.activation(ffg, ps, AF.Identity, bias=gb,
                                         scale=g_mlp[:, m:m + 1])
                    ffgs2.append(ffg)
                for half in range(2):
                    m = 2 * m2 + half
                    ffg = ffgs2[half]
                    bk4 = ptp.tile([P, 4, P], BF16, tag="tpb")
                    for jq in range(4):
                        nc.tensor.transpose(bk4[:, jq, :],
                                            ffg[:, jq * P:(jq + 1) * P],
                                            ident)
                    nc.vector.tensor_tensor(xres[:, :, m * P:(m + 1) * P],
                                            bk4,
                                            xres[:, :, m * P:(m + 1) * P],
                                            ALU.add)

        for s in range(4):
            nc.sync.dma_start(out_ap[s * P:(s + 1) * P, :], xres[:, s, :])


# --------------------------- host side --------------------------------------
_NC_CACHE = None
_LAST_INMAPS = None


def _prep_conv_w(w):
    """(D, DK, KW) grouped conv -> [128, 8, 7, 2, 128] fp8 block-diag."""
    f8 = ml_dtypes.float8_e4m3
    wr = w.reshape(H, DK, DK, KW)            # [h, out, cin, tap]
    arr = np.zeros((P, 8, 7, 2, P), np.float32)
    for hp in range(8):
        for g in range(2):
            h = 2 * hp + g
            rs = slice(g * DK, (g + 1) * DK)
            for j in range(7):
                arr[rs, hp, j, 0, rs] = wr[h, :, :, 2 * j].T * WS
                if 2 * j + 1 < KW:
                    arr[rs, hp, j, 1, rs] = wr[h, :, :, 2 * j + 1].T * WS
    return arr.astype(f8)


def _prep_mm_w(wT, nm, nj):
    """wT [K, M] contraction-major -> [128, nm, nj, 2, 128] fp8."""
    f8 = ml_dtypes.float8_e4m3
    arr = wT.reshape(nj, 2, P, nm, P).transpose(2, 3, 0, 1, 4) * WS
    return np.ascontiguousarray(arr).astype(f8)


def kernel(**inputs):
    global _NC_CACHE, _LAST_INMAPS
    if _NC_CACHE is None:
        _NC_CACHE = build_nc()
    nc = _NC_CACHE

    f32 = np.float32
    f8 = ml_dtypes.float8_e4m3
    noisy = np.asarray(inputs["noisy_feats"], f32)
    clean = np.asarray(inputs["clean_feats"], f32)
    t = np.asarray(inputs["t"], f32)
    clean_len = np.asarray(inputs["clean_lengths"]).astype(np.int64)

    assert np.all(np.asarray(inputs["ln_noisy_g"], f32) == 1.0)
    assert np.all(np.asarray(inputs["ln_noisy_b"], f32) == 0.0)

    # AdaLayerNormZero on host (0.02% of FLOPs)
    st = t * (1.0 / (1.0 + np.exp(-t, dtype=f32)))
    emb = st @ np.asarray(inputs["ada_w"], f32).T + \
        np.asarray(inputs["ada_b"], f32)
    sh_msa, sc_msa, g_msa, sh_mlp, sc_mlp, g_mlp = np.split(emb, 6, axis=1)

    wql = _prep_conv_w(np.asarray(inputs["wq"], f32))
    wkl = _prep_conv_w(np.asarray(inputs["wk"], f32))
    wvl = _prep_conv_w(np.asarray(inputs["wv"], f32))
    bfd = ml_dtypes.bfloat16
    fcw = _prep_mm_w(np.asarray(inputs["fc_w"], f32).T, 8, 4)
    w1t = np.ascontiguousarray(
        np.asarray(inputs["ff_w1"], f32).T.reshape(8, P, 32, P)
        .transpose(1, 2, 0, 3)).astype(bfd)
    w2t = np.ascontiguousarray(
        np.asarray(inputs["ff_w2"], f32).T.reshape(32, P, 8, P)
        .transpose(1, 2, 0, 3)).astype(bfd)

    common = dict(
        clng=np.asarray(inputs["ln_clean_g"], f32).copy(),
        clnb=np.asarray(inputs["ln_clean_b"], f32).copy(),
        wq=wql, wk=wkl, wv=wvl,
        bq=np.asarray(inputs["bq"], f32).copy(),
        bk=np.asarray(inputs["bk"], f32).copy(),
        bv=np.asarray(inputs["bv"], f32).copy(),
        fcw=fcw, fcb=np.asarray(inputs["fc_b"], f32).copy(),
        w1t=w1t, fb1=np.asarray(inputs["ff_b1"], f32).copy(),
        w2t=w2t, fb2=np.asarray(inputs["ff_b2"], f32).copy(),
    )

    in_maps = []
    for i in range(8):
        b, half = i // 2, i % 2
        t0 = half * TQ
        noisyH = np.zeros((NHW, D), f32)
        lo, hi = t0 - P, t0 + 640
        clo, chi = max(lo, 0), min(hi, T)
        noisyH[clo - lo:chi - lo] = noisy[b, clo:chi]
        mvec = (np.arange(T) < clean_len[b]).astype(f32)
        mk = np.broadcast_to(
            mvec.reshape(8, P).T[:, :, None, None], (P, 8, 2, DK)).astype(f8)
        eL = 0.0 if half == 0 else 1.0
        eR = 1.0 if half == 0 else 0.0
        mod = np.stack([
            sh_msa[b], 1.0 + sc_msa[b], g_msa[b],
            sh_mlp[b], 1.0 + sc_mlp[b], g_mlp[b],
            sh_msa[b] * eL, (1.0 + sc_msa[b]) * eL,
            sh_msa[b] * eR, (1.0 + sc_msa[b]) * eR,
        ]).astype(f32)
        m = dict(common)
        m.update(noisyH=noisyH, clean=clean[b].copy(), mod=mod,
                 mk=np.ascontiguousarray(mk))
        in_maps.append(m)

    _LAST_INMAPS = in_maps
    res = run_bass_kernel_spmd(nc, in_maps, core_ids=list(range(8)))
    out = np.empty((B, T, D), f32)
    for i in range(8):
        b, half = i // 2, i % 2
        out[b, half * TQ:(half + 1) * TQ] = res.results[i]["out"]
    return out


if __name__ == "__main__":
    build_nc()
    print("build ok")


# revision 3
# speedup vs baseline: 1.1759x; 1.1759x over previous
# Trainium2 Bass kernel for nn_CrossAttention_noise (B=4, T1=T2=1024, D=1024,
# H=16, DK=64, KW=13, FF=4096), SPMD over 8 NeuronCores.
#
# Sharding: core i handles batch b=i//2 and query-token half t0=(i%2)*512.
# All heavy matmuls run in fp8e4 with DoubleRow perf mode (2 K-tiles per
# pass):
#   - grouped convs: 2 heads per matmul via block-diagonal weights; the two
#     K-tiles are the shift-0/shift-1 planes of the transposed input (the
#     shift-1 plane is a SBUF->SBUF DMA copy), so one DR matmul covers 2 taps
#     x 128 channels; 7 matmuls cover the padded 14 taps.
#   - scores: lhsT/rhs use a stride-0 "pair broadcast" so the DR pass
#     computes 2*k^T q; the exp compensates with scale 1/16.
#   - PV (token-major out), fc, FFN: K-tile pairs are adjacent 128-ch blocks.
# The key-padding mask is folded into v65 (value rows and the ones-column
# multiplied by 0/1), so exp needs no per-chunk bias and one Activation op
# covers two score chunks (a 2-bank PSUM tile).
import numpy as np
import ml_dtypes
from contextlib import ExitStack

import concourse.bass as bass
import concourse.mybir as mybir
import concourse.tile as tile
from concourse import bacc
from concourse.bass_utils import run_bass_kernel_spmd
from concourse.masks import make_identity

BF16 = mybir.dt.bfloat16
F32 = mybir.dt.float32
FP8 = mybir.dt.float8e4
DR = mybir.MatmulPerfMode.DoubleRow
AF = mybir.ActivationFunctionType
ALU = mybir.AluOpType
AX = mybir.AxisListType

B, T, D, H, DK, KW, FF = 4, 1024, 1024, 16, 64, 13, 4096
TQ = 512           # query tokens per core
P = 128
NHW = 768          # noisy halo window rows (zero-padded on host)
NTW = 528          # nt2 plane width (524 used + 4 pad)
CTW = 1040         # ct2 plane width (1036 used + 4 pad)
EPS1, EPS2 = 1e-5, 1e-6
WS = 32.0          # host-side weight scale (power of two)
WSI = 1.0 / WS


def pair_b(ap2d):
    """[p, N] -> [p, 2, N] stride-0 plane broadcast (for double-q scores)."""
    p, n = ap2d.shape
    return ap2d.unsqueeze(1).broadcast_to((p, 2, n))


def build_nc():
    nc = bacc.Bacc("TRN2", target_bir_lowering=False, debug=False,
                   num_devices=8)
    dt = {}

    def din(name, shape, dtype):
        dt[name] = nc.dram_tensor(name, list(shape), dtype,
                                  kind="ExternalInput").ap()

    din("noisyH", (NHW, D), F32)       # rows [t0-128, t0+640), zero padded
    din("clean", (T, D), F32)
    din("mod", (10, D), F32)           # sh_m,1+sc_m,g_m,sh_f,1+sc_f,g_f,
    #                                     shL,(1+sc)L,shR,(1+sc)R (edge-masked)
    din("clng", (D,), F32)
    din("clnb", (D,), F32)
    din("wq", (P, 8, 7, 2, P), FP8)    # [cin][pair][tap-pair j][plane][cout]
    din("wk", (P, 8, 7, 2, P), FP8)
    din("wv", (P, 8, 7, 2, P), FP8)
    din("bq", (D,), F32)
    din("bk", (D,), F32)
    din("bv", (D,), F32)
    din("mk", (P, 8, 2, DK), FP8)      # key mask replicated (part,chunk,h2,dk)
    din("fcw", (P, 8, 4, 2, P), FP8)   # [ic][m][j][plane][oc]
    din("fcb", (D,), F32)
    din("w1h", (P, 32, 4, 2, P), FP8)  # [ic][m][j][plane][oc] hi level
    din("w1l", (P, 32, 4, 2, P), FP8)  # lo level (residual)
    din("fb1", (FF,), F32)
    din("w2h", (P, 8, 16, 2, P), FP8)
    din("w2l", (P, 8, 16, 2, P), FP8)
    din("fb2", (D,), F32)
    out_ap = nc.dram_tensor("out", [TQ, D], F32, kind="ExternalOutput").ap()

    with tile.TileContext(nc) as tc:
        _emit(tc, dt, out_ap)
    nc.compile()
    return nc


def _emit(tc, dt, out_ap):
    nc = tc.nc
    with ExitStack() as ctx:
        const = ctx.enter_context(tc.tile_pool(name="const", bufs=1))
        small = ctx.enter_context(tc.tile_pool(name="small", bufs=4))
        lnio = ctx.enter_context(tc.tile_pool(name="lnio", bufs=4))
        big = ctx.enter_context(tc.tile_pool(name="bigsb", bufs=1))
        pcv = ctx.enter_context(tc.tile_pool(name="pcv", bufs=1, space="PSUM"))
        psc = ctx.enter_context(tc.tile_pool(name="psc", bufs=2, space="PSUM"))
        ptp = ctx.enter_context(tc.tile_pool(name="ptp", bufs=2, space="PSUM"))
        ppv = ctx.enter_context(tc.tile_pool(name="ppv", bufs=1, space="PSUM"))

        ident = const.tile([P, P], BF16)
        make_identity(nc, ident)
        e_1 = const.tile([P, 1], F32)
        nc.vector.memset(e_1, EPS1)
        e_12 = const.tile([P, 1], F32)
        nc.vector.memset(e_12, EPS1 * EPS2)
        e_2 = const.tile([P, 1], F32)
        nc.vector.memset(e_2, EPS2)
        EPS_T = {"1": e_1, "12": e_12, "2": e_2}

        def chanvec(name, w=8):
            t = const.tile([P, w], F32, tag=f"cv_{name}")
            nc.sync.dma_start(t, dt[name].rearrange("(m p) -> p m", p=P))
            return t

        bq_s, bk_s, bv_s = chanvec("bq"), chanvec("bk"), chanvec("bv")
        fcb_s, fb2_s = chanvec("fcb"), chanvec("fb2")
        clng_s, clnb_s = chanvec("clng"), chanvec("clnb")
        fb1_s = chanvec("fb1", 32)
        mod_s = const.tile([P, 10, 8], F32)
        for s in range(10):
            nc.sync.dma_start(mod_s[:, s, :],
                              dt["mod"][s].rearrange("(m p) -> p m", p=P))
        sh_msa, sc_msa, g_msa = mod_s[:, 0, :], mod_s[:, 1, :], mod_s[:, 2, :]
        sh_mlp, sc_mlp, g_mlp = mod_s[:, 3, :], mod_s[:, 4, :], mod_s[:, 5, :]
        shL, scL = mod_s[:, 6, :], mod_s[:, 7, :]
        shR, scR = mod_s[:, 8, :], mod_s[:, 9, :]
        mk_s = const.tile([P, 8, 2, DK], FP8)
        nc.sync.dma_start(mk_s, dt["mk"])

        def stats(x, eng_sq):
            """(mu, var) [p,1] f32 for rows of x [p, D]."""
            s = small.tile([P, 1], F32, tag="st_s")
            nc.vector.reduce_sum(s, x, axis=AX.X)
            sq = small.tile([P, 1], F32, tag="st_sq")
            scr = small.tile([P, D], BF16, tag="st_scr", bufs=2)
            nc.scalar.activation(scr, x, AF.Square, accum_out=sq)
            mu = small.tile([P, 1], F32, tag="st_mu")
            nc.vector.tensor_scalar_mul(mu, s, 1.0 / D)
            msq = small.tile([P, 1], F32, tag="st_msq")
            nc.vector.tensor_tensor(msq, mu, mu, ALU.mult)
            var = small.tile([P, 1], F32, tag="st_var")
            nc.vector.tensor_scalar(var, sq, 1.0 / D, msq,
                                    ALU.mult, ALU.subtract)
            return mu, var

        def rs_beta(mu, var, eps_key, scale, tag):
            """(r, beta) with r = 1/sqrt(var*scale+eps), beta = -mu*r."""
            st = small.tile([P, 1], F32, tag=tag + "_s")
            nc.scalar.activation(st, var, AF.Sqrt, bias=EPS_T[eps_key],
                                 scale=scale)
            rt = small.tile([P, 1], F32, tag=tag + "_r")
            nc.vector.reciprocal(rt, st)
            bt = small.tile([P, 1], F32, tag=tag + "_b")
            nc.vector.tensor_tensor(bt, mu, rt, ALU.mult)
            nc.vector.tensor_scalar_mul(bt, bt, -1.0)
            return rt, bt

        xres = big.tile([P, 4, D], F32)        # y1 rows [t0, t0+512); then x
        abc_cm = tc.tile_pool(name="abc", bufs=1)
        abc = abc_cm.__enter__()
        nt2 = [abc.tile([P, 2, NTW], FP8, name=f"nt2_{m}") for m in range(8)]
        ct2 = [abc.tile([P, 2, CTW], FP8, name=f"ct2_{m}") for m in range(8)]
        qT = [abc.tile([P, TQ], FP8, name=f"qT_{m}") for m in range(8)]

        wqs = abc.tile([P, 8, 7, 2, P], FP8, name="wqs")
        wks = abc.tile([P, 8, 7, 2, P], FP8, name="wks")
        wvs = abc.tile([P, 8, 7, 2, P], FP8, name="wvs")

        # one-time pad zeroing (tiles are persistent)
        for m in range(8):
            nc.gpsimd.memset(nt2[m][:, 0, 524:528], 0.0)
            nc.gpsimd.memset(nt2[m][:, 1, 523:528], 0.0)
            nc.gpsimd.memset(ct2[m][:, 0, 0:6], 0.0)
            nc.gpsimd.memset(ct2[m][:, 0, 1030:1040], 0.0)
            nc.gpsimd.memset(ct2[m][:, 1, 1029:1040], 0.0)

        # ---- Phase A: noisy stats -> y1 (xres) + y2 -> nt2 + Q convs --------
        with tc.tile_pool(name="lnA", bufs=1) as lnA:
            y2 = [lnA.tile([P, D], BF16, name=f"y2_{r}") for r in range(6)]
            for r in range(6):
                xt = lnio.tile([P, D], F32, tag="ln_in", bufs=2)
                nc.sync.dma_start(xt, dt["noisyH"][r * P:(r + 1) * P, :])
                mu, var = stats(xt, "act" if r % 2 else "dve")
                # fused LN2(LN1(x)) for gamma=1,beta=0:
                #   y2 = (x-mu)/sqrt(var*(1+eps2) + eps1*eps2)
                r2, b2 = rs_beta(mu, var, "12", 1.0 + EPS2, "r2")
                nc.gpsimd.tensor_scalar(y2[r], xt, r2, b2, ALU.mult,
                                        ALU.add)
                if 1 <= r <= 4:
                    r1, b1 = rs_beta(mu, var, "1", 1.0, "r1")
                    nc.gpsimd.tensor_scalar(xres[:, r - 1, :], xt, r1, b1,
                                            ALU.mult, ALU.add)
            for m in range(8):
                bA = ptp.tile([P, 4, P], BF16, tag="tpb")
                bB = ptp.tile([P, 4, P], BF16, tag="tpb")
                for r in range(6):
                    dst = bA[:, r, :] if r < 4 else bB[:, r - 4, :]
                    nc.tensor.transpose(dst, y2[r][:, m * P:(m + 1) * P],
                                        ident)
                mm, ms = sc_msa[:, m:m + 1], sh_msa[:, m:m + 1]
                nc.vector.tensor_scalar(nt2[m][:, 0, 0:6], bA[:, 0, 122:128],
                                        scL[:, m:m + 1], shL[:, m:m + 1],
                                        ALU.mult, ALU.add)
                nc.vector.tensor_scalar(
                    nt2[m][:, 0, 6:390],
                    bA.rearrange("p a b -> p (a b)")[:, 128:512],
                    mm, ms, ALU.mult, ALU.add)
                nc.vector.tensor_scalar(nt2[m][:, 0, 390:518],
                                        bB[:, 0, :], mm, ms,
                                        ALU.mult, ALU.add)
                nc.vector.tensor_scalar(nt2[m][:, 0, 518:524], bB[:, 1, 0:6],
                                        scR[:, m:m + 1], shR[:, m:m + 1],
                                        ALU.mult, ALU.add)
                nc.sync.dma_start(nt2[m][:, 1, 0:523], nt2[m][:, 0, 1:524])
        nc.sync.dma_start(wqs, dt["wq"])
        for hp in range(8):
            ps = pcv.tile([P, TQ], F32, tag="conv")
            for j in range(7):
                nc.tensor.matmul(ps, wqs[:, hp, j],
                                 nt2[hp][:, :, 2 * j:2 * j + TQ],
                                 start=(j == 0), stop=(j == 6), perf_mode=DR)
            nc.vector.tensor_scalar(qT[hp], ps, WSI, bq_s[:, hp:hp + 1],
                                    ALU.mult, ALU.add)

        # ---- Phase B: clean stats -> y -> ct2 -------------------------------
        with tc.tile_pool(name="lnB", bufs=1) as lnB:
            yc = [lnB.tile([P, D], BF16, name=f"yc_{r}") for r in range(8)]
            for r in range(8):
                xt = lnio.tile([P, D], F32, tag="ln_in", bufs=2)
                nc.sync.dma_start(xt, dt["clean"][r * P:(r + 1) * P, :])
                mu, var = stats(xt, "act" if r % 2 else "dve")
                r1, b1 = rs_beta(mu, var, "1", 1.0, "rc")
                nc.gpsimd.tensor_scalar(yc[r], xt, r1, b1, ALU.mult,
                                        ALU.add)
            for m in range(8):
                gg, bb = clng_s[:, m:m + 1], clnb_s[:, m:m + 1]
                for half in range(2):
                    bk4 = ptp.tile([P, 4, P], BF16, tag="tpb")
                    for r4 in range(4):
                        nc.tensor.transpose(
                            bk4[:, r4, :],
                            yc[half * 4 + r4][:, m * P:(m + 1) * P], ident)
                    nc.vector.tensor_scalar(
                        ct2[m][:, 0, 6 + half * 512:518 + half * 512],
                        bk4.rearrange("p a b -> p (a b)"), gg, bb,
                        ALU.mult, ALU.add)
                nc.sync.dma_start(ct2[m][:, 1, 0:1039], ct2[m][:, 0, 1:1040])

        # ---- Phase C: per-pair K/V conv + attention -------------------------
        attp_cm = tc.tile_pool(name="attp", bufs=1)
        attp = attp_cm.__enter__()
        attnT = attp.tile([P, 8, TQ], FP8)
        attnQ = attp.tile([P, 4, H, DK], BF16)
        nc.sync.dma_start(wks, dt["wk"])
        nc.sync.dma_start(wvs, dt["wv"])
        with tc.tile_pool(name="hpool", bufs=2) as hpool:
            for hp in range(8):
                kT = hpool.tile([P, 2, TQ], FP8, tag="kT")
                for c in range(2):
                    ps = pcv.tile([P, TQ], F32, tag="conv")
                    for j in range(7):
                        nc.tensor.matmul(
                            ps, wks[:, hp, j],
                            ct2[hp][:, :, 2 * j + c * TQ:2 * j + (c + 1) * TQ],
                            start=(j == 0), stop=(j == 6), perf_mode=DR)
                    nc.vector.tensor_scalar(kT[:, c, :], ps, WSI,
                                            bk_s[:, hp:hp + 1],
                                            ALU.mult, ALU.add)
                vT = hpool.tile([P, 2, TQ], BF16, tag="vT")
                for c in range(2):
                    ps = pcv.tile([P, TQ], F32, tag="conv")
                    for j in range(7):
                        nc.tensor.matmul(
                            ps, wvs[:, hp, j],
                            ct2[hp][:, :, 2 * j + c * TQ:2 * j + (c + 1) * TQ],
                            start=(j == 0), stop=(j == 6), perf_mode=DR)
                    nc.vector.tensor_scalar(vT[:, c, :], ps, WSI,
                                            bv_s[:, hp:hp + 1],
                                            ALU.mult, ALU.add)
                # v65: token-major masked values + masked ones column
                v65 = hpool.tile([P, 8, 2, 80], FP8, tag="v65")
                nc.vector.tensor_copy(v65[:, :, :, 64], mk_s[:, :, :, 0])
                vTf = vT.rearrange("p a b -> p (a b)")
                for half in range(2):
                    bk4 = ptp.tile([P, 4, P], BF16, tag="tpb")
                    for c4 in range(4):
                        cc = half * 4 + c4
                        nc.tensor.transpose(bk4[:, c4, :],
                                            vTf[:, cc * P:(cc + 1) * P],
                                            ident)
                    nc.vector.tensor_tensor(
                        v65[:, half * 4:half * 4 + 4, :, 0:64],
                        bk4.rearrange("p a (h d) -> p a h d", h=2),
                        mk_s[:, half * 4:half * 4 + 4, :, :], ALU.mult)
                kTf = kT.rearrange("p a b -> p (a b)")
                for hh in range(2):
                    h = 2 * hp + hh
                    rows = slice(hh * DK, (hh + 1) * DK)
                    pT = hpool.tile([P, 8, TQ], FP8, tag="pT")
                    for c2 in range(4):
                        ps = psc.tile([P, 2, TQ], F32, tag="sc")
                        for c in range(2):
                            cc = 2 * c2 + c
                            nc.tensor.matmul(
                                ps[:, c, :],
                                pair_b(kTf[rows, cc * P:(cc + 1) * P]),
                                pair_b(qT[hp][rows, :]),
                                start=True, stop=True, perf_mode=DR)
                        nc.scalar.activation(
                            pT[:, 2 * c2:2 * c2 + 2, :].rearrange(
                                "p a b -> p (a b)"),
                            ps.rearrange("p a b -> p (a b)"), AF.Exp,
                            scale=0.0625)
                    pv = ppv.tile([P, 4, 65], F32, tag="pv")
                    for jq in range(4):
                        for c2 in range(4):
                            nc.tensor.matmul(
                                pv[:, jq, :],
                                pT[:, 2 * c2:2 * c2 + 2, jq * P:(jq + 1) * P],
                                v65[:, 2 * c2:2 * c2 + 2, hh, 0:65],
                                start=(c2 == 0), stop=(c2 == 3), perf_mode=DR)
                    rec = small.tile([P, 4], F32, tag="rec")
                    nc.vector.reciprocal(rec, pv[:, :, 64])
                    for jq in range(4):
                        nc.vector.tensor_scalar_mul(attnQ[:, jq, h, :],
                                                    pv[:, jq, 0:64],
                                                    rec[:, jq:jq + 1])

        abc_cm.__exit__(None, None, None)

        # ---- Phase D: attnT transposes + fc + residual ----------------------
        aqf = attnQ.rearrange("p a h d -> p (a h d)")
        for hp in range(8):
            bk4 = ptp.tile([P, 4, P], BF16, tag="tpb")
            for jq in range(4):
                nc.tensor.transpose(
                    bk4[:, jq, :],
                    aqf[:, jq * 1024 + hp * P:jq * 1024 + (hp + 1) * P],
                    ident)
            nc.vector.tensor_copy(attnT[:, hp, :],
                                  bk4.rearrange("p a b -> p (a b)"))
        fcws = attp.tile([P, 8, 4, 2, P], FP8, name="fcws")
        nc.sync.dma_start(fcws, dt["fcw"])
        for m2 in range(4):
            ps2 = psc.tile([P, 2, TQ], F32, tag="sc")
            for half in range(2):
                for j in range(4):
                    nc.tensor.matmul(ps2[:, half, :],
                                     fcws[:, 2 * m2 + half, j],
                                     attnT[:, 2 * j:2 * j + 2, :],
                                     start=(j == 0), stop=(j == 3),
                                     perf_mode=DR)
          # per-m evacs below read ps2 planes
            fcgs2 = []
            for half in range(2):
                m = 2 * m2 + half
                ps = ps2[:, half, :]
                fcg = lnio.tile([P, TQ], BF16, tag="fcg", bufs=4)
                gb = small.tile([P, 1], F32, tag="gb")
                nc.vector.tensor_tensor(gb, fcb_s[:, m:m + 1],
                                        g_msa[:, m:m + 1], ALU.mult)
                gsc = small.tile([P, 1], F32, tag="gsc")
                nc.vector.tensor_scalar_mul(gsc, g_msa[:, m:m + 1], WSI)
                nc.scalar.activation(fcg, ps, AF.Identity, bias=gb, scale=gsc)
                fcgs2.append(fcg)
            for half in range(2):
                m = 2 * m2 + half
                fcg = fcgs2[half]
                bk4 = ptp.tile([P, 4, P], BF16, tag="tpb")
                for jq in range(4):
                    nc.tensor.transpose(bk4[:, jq, :],
                                        fcg[:, jq * P:(jq + 1) * P], ident)
                nc.vector.tensor_tensor(xres[:, :, m * P:(m + 1) * P], bk4,
                                        xres[:, :, m * P:(m + 1) * P],
                                        ALU.add)

        # ---- Phase E: LN3 + mlp modulation -> n2T ---------------------------
        attp_cm.__exit__(None, None, None)
        abc_cm.__exit__(None, None, None)
        n2p_cm = tc.tile_pool(name="n2p", bufs=1)
        n2p = n2p_cm.__enter__()
        n2hi = n2p.tile([P, 8, TQ], FP8)
        n2lo = n2p.tile([P, 8, TQ], FP8)
        with tc.tile_pool(name="lnE", bufs=1) as lnE:
            l3 = [lnE.tile([P, D], BF16, name=f"l3_{s}") for s in range(4)]
            for s in range(4):
                mu, var = stats(xres[:, s, :], "act" if s % 2 else "dve")
                r3, b3 = rs_beta(mu, var, "2", 1.0, "r3")
                nc.gpsimd.tensor_scalar(l3[s], xres[:, s, :], r3, b3,
                                        ALU.mult, ALU.add)
            for m in range(8):
                bk4 = ptp.tile([P, 4, P], BF16, tag="tpb")
                for s in range(4):
                    nc.tensor.transpose(bk4[:, s, :],
                                        l3[s][:, m * P:(m + 1) * P], ident)
                n2bf = lnio.tile([P, TQ], BF16, tag="n2bf", bufs=2)
                nc.vector.tensor_scalar(n2bf,
                                        bk4.rearrange("p a b -> p (a b)"),
                                        sc_mlp[:, m:m + 1],
                                        sh_mlp[:, m:m + 1],
                                        ALU.mult, ALU.add)
                nc.gpsimd.tensor_copy(n2hi[:, m, :], n2bf)
                nc.vector.tensor_tensor(n2lo[:, m, :], n2bf, n2hi[:, m, :],
                                        ALU.subtract)

        # ---- Phase F: FFN ---------------------------------------------------
        with tc.tile_pool(name="ffp", bufs=1) as ffp:
            ffahi = ffp.tile([P, 32, TQ], FP8)
            ffalo = ffp.tile([P, 32, TQ], FP8)
            for m8 in range(4):
                w1ch = ffp.tile([P, 8, 4, 2, P], FP8, tag="w1ch", bufs=2)
                nc.sync.dma_start(w1ch, dt["w1h"][:, m8 * 8:(m8 + 1) * 8])
                w1cl = ffp.tile([P, 8, 4, 2, P], FP8, tag="w1cl", bufs=2)
                nc.sync.dma_start(w1cl, dt["w1l"][:, m8 * 8:(m8 + 1) * 8])
                for m2 in range(4):
                    ps2 = psc.tile([P, 2, TQ], F32, tag="sc")
                    for half in range(2):
                        m_ = 2 * m2 + half
                        m = m8 * 8 + m_
                        first = True
                        for j in range(4):
                            xh = n2hi[:, 2 * j:2 * j + 2, :]
                            xl = n2lo[:, 2 * j:2 * j + 2, :]
                            for wgt, rhs in ((w1ch, xh), (w1ch, xl),
                                             (w1cl, xh)):
                                nc.tensor.matmul(
                                    ps2[:, half, :], wgt[:, m_, j], rhs,
                                    start=first,
                                    stop=(j == 3 and rhs is xh
                                          and wgt is w1cl),
                                    perf_mode=DR)
                                first = False
                        ffabf = lnio.tile([P, TQ], BF16, tag="ffabf",
                                          bufs=3)
                        nc.scalar.activation(ffabf, ps2[:, half, :],
                                             AF.Gelu_apprx_tanh,
                                             bias=fb1_s[:, m:m + 1],
                                             scale=WSI)
                        nc.gpsimd.tensor_copy(ffahi[:, m, :], ffabf)
                        eng = nc.vector if m % 2 else nc.gpsimd
                        eng.tensor_tensor(ffalo[:, m, :], ffabf,
                                          ffahi[:, m, :], ALU.subtract)
            for m2 in range(4):
                ps2 = psc.tile([P, 2, TQ], F32, tag="sc")
                ffgs2 = []
                for half in range(2):
                    m = 2 * m2 + half
                    w2ch = ffp.tile([P, 16, 2, P], FP8, tag="w2ch", bufs=3)
                    nc.sync.dma_start(w2ch, dt["w2h"][:, m])
                    w2cl = ffp.tile([P, 16, 2, P], FP8, tag="w2cl", bufs=3)
                    nc.sync.dma_start(w2cl, dt["w2l"][:, m])
                    first = True
                    for j in range(16):
                        xh = ffahi[:, 2 * j:2 * j + 2, :]
                        xl = ffalo[:, 2 * j:2 * j + 2, :]
                        for wgt, rhs in ((w2ch, xh), (w2ch, xl), (w2cl, xh)):
                            nc.tensor.matmul(
                                ps2[:, half, :], wgt[:, j], rhs,
                                start=first,
                                stop=(j == 15 and rhs is xh and wgt is w2cl),
                                perf_mode=DR)
                            first = False
                    ps = ps2[:, half, :]
                    ffg = lnio.tile([P, TQ], BF16, tag="ffg", bufs=4)
                    gb = small.tile([P, 1], F32, tag="gb2")
                    nc.vector.tensor_tensor(gb, fb2_s[:, m:m + 1],
                                            g_mlp[:, m:m + 1], ALU.mult)
                    gs2 = small.tile([P, 1], F32, tag="gs2")
                    nc.vector.tensor_scalar_mul(gs2, g_mlp[:, m:m + 1], WSI)
                    nc.scalar.activation(ffg, ps, AF.Identity, bias=gb,
                                         scale=gs2)
                    ffgs2.append(ffg)
                for half in range(2):
                    m = 2 * m2 + half
                    ffg = ffgs2[half]
                    bk4 = ptp.tile([P, 4, P], BF16, tag="tpb")
                    for jq in range(4):
                        nc.tensor.transpose(bk4[:, jq, :],
                                            ffg[:, jq * P:(jq + 1) * P],
                                            ident)
                    nc.vector.tensor_tensor(xres[:, :, m * P:(m + 1) * P],
                                            bk4,
                                            xres[:, :, m * P:(m + 1) * P],
                                            ALU.add)

        n2p_cm.__exit__(None, None, None)
        for s in range(4):
            nc.sync.dma_start(out_ap[s * P:(s + 1) * P, :], xres[:, s, :])


# --------------------------- host side --------------------------------------
_NC_CACHE = None
_LAST_INMAPS = None


def _prep_conv_w(w):
    """(D, DK, KW) grouped conv -> [128, 8, 7, 2, 128] fp8 block-diag."""
    f8 = ml_dtypes.float8_e4m3
    wr = w.reshape(H, DK, DK, KW)            # [h, out, cin, tap]
    arr = np.zeros((P, 8, 7, 2, P), np.float32)
    for hp in range(8):
        for g in range(2):
            h = 2 * hp + g
            rs = slice(g * DK, (g + 1) * DK)
            for j in range(7):
                arr[rs, hp, j, 0, rs] = wr[h, :, :, 2 * j].T * WS
                if 2 * j + 1 < KW:
                    arr[rs, hp, j, 1, rs] = wr[h, :, :, 2 * j + 1].T * WS
    return arr.astype(f8)


def _prep_mm_w(wT, nm, nj):
    """wT [K, M] contraction-major -> [128, nm, nj, 2, 128] fp8."""
    f8 = ml_dtypes.float8_e4m3
    arr = wT.reshape(nj, 2, P, nm, P).transpose(2, 3, 0, 1, 4) * WS
    return np.ascontiguousarray(arr).astype(f8)


def kernel(**inputs):
    global _NC_CACHE, _LAST_INMAPS
    if _NC_CACHE is None:
        _NC_CACHE = build_nc()
    nc = _NC_CACHE

    f32 = np.float32
    f8 = ml_dtypes.float8_e4m3
    noisy = np.asarray(inputs["noisy_feats"], f32)
    clean = np.asarray(inputs["clean_feats"], f32)
    t = np.asarray(inputs["t"], f32)
    clean_len = np.asarray(inputs["clean_lengths"]).astype(np.int64)

    assert np.all(np.asarray(inputs["ln_noisy_g"], f32) == 1.0)
    assert np.all(np.asarray(inputs["ln_noisy_b"], f32) == 0.0)

    # AdaLayerNormZero on host (0.02% of FLOPs)
    st = t * (1.0 / (1.0 + np.exp(-t, dtype=f32)))
    emb = st @ np.asarray(inputs["ada_w"], f32).T + \
        np.asarray(inputs["ada_b"], f32)
    sh_msa, sc_msa, g_msa, sh_mlp, sc_mlp, g_mlp = np.split(emb, 6, axis=1)

    wql = _prep_conv_w(np.asarray(inputs["wq"], f32))
    wkl = _prep_conv_w(np.asarray(inputs["wk"], f32))
    wvl = _prep_conv_w(np.asarray(inputs["wv"], f32))
    fcw = _prep_mm_w(np.asarray(inputs["fc_w"], f32).T, 8, 4)

    def hilo(wT, nm, nj):
        ws = wT * WS
        hi = ws.astype(f8).astype(f32)
        lo = ws - hi
        return (_prep_mm_w(hi / WS, nm, nj), _prep_mm_w(lo / WS, nm, nj))

    w1h, w1l = hilo(np.asarray(inputs["ff_w1"], f32).T, 32, 4)
    w2h, w2l = hilo(np.asarray(inputs["ff_w2"], f32).T, 8, 16)

    common = dict(
        clng=np.asarray(inputs["ln_clean_g"], f32).copy(),
        clnb=np.asarray(inputs["ln_clean_b"], f32).copy(),
        wq=wql, wk=wkl, wv=wvl,
        bq=np.asarray(inputs["bq"], f32).copy(),
        bk=np.asarray(inputs["bk"], f32).copy(),
        bv=np.asarray(inputs["bv"], f32).copy(),
        fcw=fcw, fcb=np.asarray(inputs["fc_b"], f32).copy(),
        w1h=w1h, w1l=w1l, fb1=np.asarray(inputs["ff_b1"], f32).copy(),
        w2h=w2h, w2l=w2l, fb2=np.asarray(inputs["ff_b2"], f32).copy(),
    )

    in_maps = []
    for i in range(8):
        b, half = i // 2, i % 2
        t0 = half * TQ
        noisyH = np.zeros((NHW, D), f32)
        lo, hi = t0 - P, t0 + 640
        clo, chi = max(lo, 0), min(hi, T)
        noisyH[clo - lo:chi - lo] = noisy[b, clo:chi]
        mvec = (np.arange(T) < clean_len[b]).astype(f32)
        mk = np.broadcast_to(
            mvec.reshape(8, P).T[:, :, None, None], (P, 8, 2, DK)).astype(f8)
        eL = 0.0 if half == 0 else 1.0
        eR = 1.0 if half == 0 else 0.0
        mod = np.stack([
            sh_msa[b], 1.0 + sc_msa[b], g_msa[b],
            sh_mlp[b], 1.0 + sc_mlp[b], g_mlp[b],
            sh_msa[b] * eL, (1.0 + sc_msa[b]) * eL,
            sh_msa[b] * eR, (1.0 + sc_msa[b]) * eR,
        ]).astype(f32)
        m = dict(common)
        m.update(noisyH=noisyH, clean=clean[b].copy(), mod=mod,
                 mk=np.ascontiguousarray(mk))
        in_maps.append(m)

    _LAST_INMAPS = in_maps
    res = run_bass_kernel_spmd(nc, in_maps, core_ids=list(range(8)))
    out = np.empty((B, T, D), f32)
    for i in range(8):
        b, half = i // 2, i % 2
        out[b, half * TQ:(half + 1) * TQ] = res.results[i]["out"]
    return out


if __name__ == "__main__":
    build_nc()
    print("build ok")


# revision 5
# speedup vs baseline: 1.1951x; 1.0163x over previous
# Trainium2 Bass kernel for nn_CrossAttention_noise (B=4, T1=T2=1024, D=1024,
# H=16, DK=64, KW=13, FF=4096), SPMD over 8 NeuronCores.
#
# Sharding: core i handles batch b=i//2 and query-token half t0=(i%2)*512.
# All heavy matmuls run in fp8e4 with DoubleRow perf mode (2 K-tiles per
# pass):
#   - grouped convs: 2 heads per matmul via block-diagonal weights; the two
#     K-tiles are the shift-0/shift-1 planes of the transposed input (the
#     shift-1 plane is a SBUF->SBUF DMA copy), so one DR matmul covers 2 taps
#     x 128 channels; 7 matmuls cover the padded 14 taps.
#   - scores: lhsT/rhs use a stride-0 "pair broadcast" so the DR pass
#     computes 2*k^T q; the exp compensates with scale 1/16.
#   - PV (token-major out), fc, FFN: K-tile pairs are adjacent 128-ch blocks.
# The key-padding mask is folded into v65 (value rows and the ones-column
# multiplied by 0/1), so exp needs no per-chunk bias and one Activation op
# covers two score chunks (a 2-bank PSUM tile).
import numpy as np
import ml_dtypes
from contextlib import ExitStack

import concourse.bass as bass
import concourse.mybir as mybir
import concourse.tile as tile
from concourse import bacc
from concourse.bass_utils import run_bass_kernel_spmd
from concourse.masks import make_identity

BF16 = mybir.dt.bfloat16
F32 = mybir.dt.float32
FP8 = mybir.dt.float8e4
DR = mybir.MatmulPerfMode.DoubleRow
AF = mybir.ActivationFunctionType
ALU = mybir.AluOpType
AX = mybir.AxisListType

B, T, D, H, DK, KW, FF = 4, 1024, 1024, 16, 64, 13, 4096
TQ = 512           # query tokens per core
P = 128
NHW = 768          # noisy halo window rows (zero-padded on host)
NTW = 528          # nt2 plane width (524 used + 4 pad)
CTW = 1040         # ct2 plane width (1036 used + 4 pad)
EPS1, EPS2 = 1e-5, 1e-6
WS = 32.0          # host-side weight scale (power of two)
WSI = 1.0 / WS


def pair_b(ap2d):
    """[p, N] -> [p, 2, N] stride-0 plane broadcast (for double-q scores)."""
    p, n = ap2d.shape
    return ap2d.unsqueeze(1).broadcast_to((p, 2, n))


def build_nc():
    nc = bacc.Bacc("TRN2", target_bir_lowering=False, debug=False,
                   num_devices=8)
    dt = {}

    def din(name, shape, dtype):
        dt[name] = nc.dram_tensor(name, list(shape), dtype,
                                  kind="ExternalInput").ap()

    din("noisyH", (NHW, D), BF16)      # rows [t0-128, t0+640), zero padded
    din("clean", (T, D), BF16)
    din("lnst", (36, P), F32)          # host LN stats: r2[6],b2[6],r1[4],
    #                                     b1[4], rc[8],bc[8]
    din("mod", (10, D), F32)           # sh_m,1+sc_m,g_m,sh_f,1+sc_f,g_f,
    #                                     shL,(1+sc)L,shR,(1+sc)R (edge-masked)
    din("clng", (D,), F32)
    din("clnb", (D,), F32)
    din("wq", (P, 8, 7, 2, P), FP8)    # [cin][pair][tap-pair j][plane][cout]
    din("wk", (P, 8, 7, 2, P), FP8)
    din("wv", (P, 8, 7, 2, P), FP8)
    din("bq", (D,), F32)
    din("bk", (D,), F32)
    din("bv", (D,), F32)
    din("mk", (P, 8, 2, DK), FP8)      # key mask replicated (part,chunk,h2,dk)
    din("fcw", (P, 8, 4, 2, P), FP8)   # [ic][m][j][plane][oc]
    din("fcb", (D,), F32)
    din("w1h", (P, 32, 4, 2, P), FP8)  # [ic][m][j][plane][oc] hi level
    din("w1l", (P, 32, 4, 2, P), FP8)  # lo level (residual)
    din("fb1", (FF,), F32)
    din("w2h", (P, 8, 16, 2, P), FP8)
    din("w2l", (P, 8, 16, 2, P), FP8)
    din("fb2", (D,), F32)
    out_ap = nc.dram_tensor("out", [TQ, D], F32, kind="ExternalOutput").ap()

    with tile.TileContext(nc) as tc:
        _emit(tc, dt, out_ap)
    nc.compile()
    return nc


def _emit(tc, dt, out_ap):
    nc = tc.nc
    with ExitStack() as ctx:
        const = ctx.enter_context(tc.tile_pool(name="const", bufs=1))
        small = ctx.enter_context(tc.tile_pool(name="small", bufs=4))
        lnio = ctx.enter_context(tc.tile_pool(name="lnio", bufs=4))
        big = ctx.enter_context(tc.tile_pool(name="bigsb", bufs=1))
        fstr = ctx.enter_context(tc.tile_pool(name="fstr", bufs=2))
        pcv = ctx.enter_context(tc.tile_pool(name="pcv", bufs=1, space="PSUM"))
        psc = ctx.enter_context(tc.tile_pool(name="psc", bufs=2, space="PSUM"))
        ptp = ctx.enter_context(tc.tile_pool(name="ptp", bufs=2, space="PSUM"))
        ppv = ctx.enter_context(tc.tile_pool(name="ppv", bufs=1, space="PSUM"))

        ident = const.tile([P, P], BF16)
        make_identity(nc, ident)
        e_1 = const.tile([P, 1], F32)
        nc.vector.memset(e_1, EPS1)
        e_12 = const.tile([P, 1], F32)
        nc.vector.memset(e_12, EPS1 * EPS2)
        e_2 = const.tile([P, 1], F32)
        nc.vector.memset(e_2, EPS2)
        EPS_T = {"1": e_1, "12": e_12, "2": e_2}

        def chanvec(name, w=8):
            t = const.tile([P, w], F32, tag=f"cv_{name}")
            nc.sync.dma_start(t, dt[name].rearrange("(m p) -> p m", p=P))
            return t

        bq_s, bk_s, bv_s = chanvec("bq"), chanvec("bk"), chanvec("bv")
        fcb_s, fb2_s = chanvec("fcb"), chanvec("fb2")
        clng_s, clnb_s = chanvec("clng"), chanvec("clnb")
        fb1_s = chanvec("fb1", 32)
        mod_s = const.tile([P, 10, 8], F32)
        for s in range(10):
            nc.sync.dma_start(mod_s[:, s, :],
                              dt["mod"][s].rearrange("(m p) -> p m", p=P))
        sh_msa, sc_msa, g_msa = mod_s[:, 0, :], mod_s[:, 1, :], mod_s[:, 2, :]
        sh_mlp, sc_mlp, g_mlp = mod_s[:, 3, :], mod_s[:, 4, :], mod_s[:, 5, :]
        shL, scL = mod_s[:, 6, :], mod_s[:, 7, :]
        shR, scR = mod_s[:, 8, :], mod_s[:, 9, :]
        mk_s = const.tile([P, 8, 2, DK], FP8)
        nc.sync.dma_start(mk_s, dt["mk"])
        lnst_s = const.tile([P, 36], F32)
        nc.sync.dma_start(lnst_s, dt["lnst"].rearrange("n p -> p n"))

        def stats(x, eng_sq):
            """(mu, var) [p,1] f32 for rows of x [p, D]."""
            s = small.tile([P, 1], F32, tag="st_s")
            nc.vector.reduce_sum(s, x, axis=AX.X)
            sq = small.tile([P, 1], F32, tag="st_sq")
            scr = small.tile([P, D], BF16, tag="st_scr", bufs=2)
            nc.scalar.activation(scr, x, AF.Square, accum_out=sq)
            mu = small.tile([P, 1], F32, tag="st_mu")
            nc.vector.tensor_scalar_mul(mu, s, 1.0 / D)
            msq = small.tile([P, 1], F32, tag="st_msq")
            nc.vector.tensor_tensor(msq, mu, mu, ALU.mult)
            var = small.tile([P, 1], F32, tag="st_var")
            nc.vector.tensor_scalar(var, sq, 1.0 / D, msq,
                                    ALU.mult, ALU.subtract)
            return mu, var

        def rs_beta(mu, var, eps_key, scale, tag):
            """(r, beta) with r = 1/sqrt(var*scale+eps), beta = -mu*r."""
            st = small.tile([P, 1], F32, tag=tag + "_s")
            nc.scalar.activation(st, var, AF.Sqrt, bias=EPS_T[eps_key],
                                 scale=scale)
            rt = small.tile([P, 1], F32, tag=tag + "_r")
            nc.vector.reciprocal(rt, st)
            bt = small.tile([P, 1], F32, tag=tag + "_b")
            nc.vector.tensor_tensor(bt, mu, rt, ALU.mult)
            nc.vector.tensor_scalar_mul(bt, bt, -1.0)
            return rt, bt

        xres = big.tile([P, 4, D], F32)        # y1 rows [t0, t0+512); then x
        abc_cm = tc.tile_pool(name="abc", bufs=1)
        abc = abc_cm.__enter__()
        nt2 = [abc.tile([P, 2, NTW], FP8, name=f"nt2_{m}") for m in range(8)]
        ct2 = [abc.tile([P, 2, CTW], FP8, name=f"ct2_{m}") for m in range(8)]
        qT = [abc.tile([P, TQ], FP8, name=f"qT_{m}") for m in range(8)]


        # one-time pad zeroing (tiles are persistent)
        for m in range(8):
            nc.gpsimd.memset(nt2[m][:, 0, 524:528], 0.0)
            nc.gpsimd.memset(nt2[m][:, 1, 523:528], 0.0)
            nc.gpsimd.memset(ct2[m][:, 0, 0:6], 0.0)
            nc.gpsimd.memset(ct2[m][:, 0, 1030:1040], 0.0)
            nc.gpsimd.memset(ct2[m][:, 1, 1029:1040], 0.0)

        # ---- Phase A: noisy stats -> y1 (xres) + y2 -> nt2 + Q convs --------
        with tc.tile_pool(name="lnA", bufs=1) as lnA:
            y2 = [lnA.tile([P, D], BF16, name=f"y2_{r}") for r in range(6)]
            for r in range(6):
                xt = lnio.tile([P, D], BF16, tag="ln_in", bufs=3)
                nc.sync.dma_start(xt, dt["noisyH"][r * P:(r + 1) * P, :])
                nc.vector.tensor_scalar(y2[r], xt, lnst_s[:, r:r + 1],
                                        lnst_s[:, 6 + r:7 + r],
                                        ALU.mult, ALU.add)
                if 1 <= r <= 4:
                    nc.gpsimd.tensor_scalar(xres[:, r - 1, :], xt,
                                            lnst_s[:, 11 + r:12 + r],
                                            lnst_s[:, 15 + r:16 + r],
                                            ALU.mult, ALU.add)
            for m in range(8):
                bA = ptp.tile([P, 4, P], BF16, tag="tpb")
                bB = ptp.tile([P, 4, P], BF16, tag="tpb")
                for r in range(6):
                    dst = bA[:, r, :] if r < 4 else bB[:, r - 4, :]
                    nc.tensor.transpose(dst, y2[r][:, m * P:(m + 1) * P],
                                        ident)
                mm, ms = sc_msa[:, m:m + 1], sh_msa[:, m:m + 1]
                nc.vector.tensor_scalar(nt2[m][:, 0, 0:6], bA[:, 0, 122:128],
                                        scL[:, m:m + 1], shL[:, m:m + 1],
                                        ALU.mult, ALU.add)
                nc.vector.tensor_scalar(
                    nt2[m][:, 0, 6:390],
                    bA.rearrange("p a b -> p (a b)")[:, 128:512],
                    mm, ms, ALU.mult, ALU.add)
                nc.vector.tensor_scalar(nt2[m][:, 0, 390:518],
                                        bB[:, 0, :], mm, ms,
                                        ALU.mult, ALU.add)
                nc.vector.tensor_scalar(nt2[m][:, 0, 518:524], bB[:, 1, 0:6],
                                        scR[:, m:m + 1], shR[:, m:m + 1],
                                        ALU.mult, ALU.add)
                nc.sync.dma_start(nt2[m][:, 1, 0:523], nt2[m][:, 0, 1:524])
        for hp in range(8):
            if hp % 4 == 0:
                wqs = abc.tile([P, 4, 7, 2, P], FP8, tag="wqs", bufs=2)
                nc.sync.dma_start(wqs, dt["wq"][:, hp:hp + 4])
            ps = pcv.tile([P, TQ], F32, tag="conv")
            for j in range(7):
                nc.tensor.matmul(ps, wqs[:, hp % 4, j],
                                 nt2[hp][:, :, 2 * j:2 * j + TQ],
                                 start=(j == 0), stop=(j == 6), perf_mode=DR)
            nc.vector.tensor_scalar(qT[hp], ps, WSI, bq_s[:, hp:hp + 1],
                                    ALU.mult, ALU.add)

        # ---- Phase B: clean stats -> y -> ct2 -------------------------------
        with tc.tile_pool(name="lnB", bufs=1) as lnB:
            yc = [lnB.tile([P, D], BF16, name=f"yc_{r}") for r in range(8)]
            for r in range(8):
                xt = lnio.tile([P, D], BF16, tag="ln_in", bufs=3)
                nc.sync.dma_start(xt, dt["clean"][r * P:(r + 1) * P, :])
                nc.vector.tensor_scalar(yc[r], xt, lnst_s[:, 20 + r:21 + r],
                                        lnst_s[:, 28 + r:29 + r],
                                        ALU.mult, ALU.add)
            for m in range(8):
                gg, bb = clng_s[:, m:m + 1], clnb_s[:, m:m + 1]
                for half in range(2):
                    bk4 = ptp.tile([P, 4, P], BF16, tag="tpb")
                    for r4 in range(4):
                        nc.tensor.transpose(
                            bk4[:, r4, :],
                            yc[half * 4 + r4][:, m * P:(m + 1) * P], ident)
                    nc.vector.tensor_scalar(
                        ct2[m][:, 0, 6 + half * 512:518 + half * 512],
                        bk4.rearrange("p a b -> p (a b)"), gg, bb,
                        ALU.mult, ALU.add)
                nc.sync.dma_start(ct2[m][:, 1, 0:1039], ct2[m][:, 0, 1:1040])

        # ---- Phase C: per-pair K/V conv + attention -------------------------
        attp_cm = tc.tile_pool(name="attp", bufs=1)
        attp = attp_cm.__enter__()
        attnT = attp.tile([P, 8, TQ], FP8)
        attnQ = attp.tile([P, 4, H, DK], BF16)
        with tc.tile_pool(name="hpool", bufs=2) as hpool:
            for hp in range(8):
                if hp % 4 == 0:
                    wks = abc.tile([P, 4, 7, 2, P], FP8, tag="wks", bufs=2)
                    nc.sync.dma_start(wks, dt["wk"][:, hp:hp + 4])
                    wvs = abc.tile([P, 4, 7, 2, P], FP8, tag="wvs", bufs=2)
                    nc.sync.dma_start(wvs, dt["wv"][:, hp:hp + 4])
                kT = hpool.tile([P, 2, TQ], FP8, tag="kT")
                for c in range(2):
                    ps = pcv.tile([P, TQ], F32, tag="conv")
                    for j in range(7):
                        nc.tensor.matmul(
                            ps, wks[:, hp % 4, j],
                            ct2[hp][:, :, 2 * j + c * TQ:2 * j + (c + 1) * TQ],
                            start=(j == 0), stop=(j == 6), perf_mode=DR)
                    nc.vector.tensor_scalar(kT[:, c, :], ps, WSI,
                                            bk_s[:, hp:hp + 1],
                                            ALU.mult, ALU.add)
                vT = hpool.tile([P, 2, TQ], BF16, tag="vT")
                for c in range(2):
                    ps = pcv.tile([P, TQ], F32, tag="conv")
                    for j in range(7):
                        nc.tensor.matmul(
                            ps, wvs[:, hp % 4, j],
                            ct2[hp][:, :, 2 * j + c * TQ:2 * j + (c + 1) * TQ],
                            start=(j == 0), stop=(j == 6), perf_mode=DR)
                    nc.vector.tensor_scalar(vT[:, c, :], ps, WSI,
                                            bv_s[:, hp:hp + 1],
                                            ALU.mult, ALU.add)
                # v65: token-major masked values + masked ones column
                v65 = hpool.tile([P, 8, 2, 80], FP8, tag="v65")
                nc.vector.tensor_copy(v65[:, :, :, 64], mk_s[:, :, :, 0])
                vTf = vT.rearrange("p a b -> p (a b)")
                for half in range(2):
                    bk4 = ptp.tile([P, 4, P], BF16, tag="tpb")
                    for c4 in range(4):
                        cc = half * 4 + c4
                        nc.tensor.transpose(bk4[:, c4, :],
                                            vTf[:, cc * P:(cc + 1) * P],
                                            ident)
                    nc.vector.tensor_tensor(
                        v65[:, half * 4:half * 4 + 4, :, 0:64],
                        bk4.rearrange("p a (h d) -> p a h d", h=2),
                        mk_s[:, half * 4:half * 4 + 4, :, :], ALU.mult)
                kTf = kT.rearrange("p a b -> p (a b)")
                for hh in range(2):
                    h = 2 * hp + hh
                    rows = slice(hh * DK, (hh + 1) * DK)
                    pT = hpool.tile([P, 8, TQ], FP8, tag="pT")
                    for c2 in range(4):
                        ps = psc.tile([P, 2, TQ], F32, tag="sc")
                        for c in range(2):
                            cc = 2 * c2 + c
                            nc.tensor.matmul(
                                ps[:, c, :],
                                pair_b(kTf[rows, cc * P:(cc + 1) * P]),
                                pair_b(qT[hp][rows, :]),
                                start=True, stop=True, perf_mode=DR)
                        nc.scalar.activation(
                            pT[:, 2 * c2:2 * c2 + 2, :].rearrange(
                                "p a b -> p (a b)"),
                            ps.rearrange("p a b -> p (a b)"), AF.Exp,
                            scale=0.0625)
                    pv = ppv.tile([P, 4, 65], F32, tag="pv")
                    for jq in range(4):
                        for c2 in range(4):
                            nc.tensor.matmul(
                                pv[:, jq, :],
                                pT[:, 2 * c2:2 * c2 + 2, jq * P:(jq + 1) * P],
                                v65[:, 2 * c2:2 * c2 + 2, hh, 0:65],
                                start=(c2 == 0), stop=(c2 == 3), perf_mode=DR)
                    rec = small.tile([P, 4], F32, tag="rec")
                    nc.vector.reciprocal(rec, pv[:, :, 64])
                    for jq in range(4):
                        nc.vector.tensor_scalar_mul(attnQ[:, jq, h, :],
                                                    pv[:, jq, 0:64],
                                                    rec[:, jq:jq + 1])

        abc_cm.__exit__(None, None, None)

        # ---- Phase D: attnT transposes + fc + residual ----------------------
        aqf = attnQ.rearrange("p a h d -> p (a h d)")
        for hp in range(8):
            bk4 = ptp.tile([P, 4, P], BF16, tag="tpb")
            for jq in range(4):
                nc.tensor.transpose(
                    bk4[:, jq, :],
                    aqf[:, jq * 1024 + hp * P:jq * 1024 + (hp + 1) * P],
                    ident)
            nc.vector.tensor_copy(attnT[:, hp, :],
                                  bk4.rearrange("p a b -> p (a b)"))
        fcws = attp.tile([P, 8, 4, 2, P], FP8, name="fcws")
        nc.sync.dma_start(fcws, dt["fcw"])
        for m2 in range(4):
            ps2 = psc.tile([P, 2, TQ], F32, tag="sc")
            for half in range(2):
                for j in range(4):
                    nc.tensor.matmul(ps2[:, half, :],
                                     fcws[:, 2 * m2 + half, j],
                                     attnT[:, 2 * j:2 * j + 2, :],
                                     start=(j == 0), stop=(j == 3),
                                     perf_mode=DR)
          # per-m evacs below read ps2 planes
            fcgs2 = []
            for half in range(2):
                m = 2 * m2 + half
                ps = ps2[:, half, :]
                fcg = lnio.tile([P, TQ], BF16, tag="fcg", bufs=4)
                gb = small.tile([P, 1], F32, tag="gb")
                nc.vector.tensor_tensor(gb, fcb_s[:, m:m + 1],
                                        g_msa[:, m:m + 1], ALU.mult)
                gsc = small.tile([P, 1], F32, tag="gsc")
                nc.vector.tensor_scalar_mul(gsc, g_msa[:, m:m + 1], WSI)
                nc.scalar.activation(fcg, ps, AF.Identity, bias=gb, scale=gsc)
                fcgs2.append(fcg)
            for half in range(2):
                m = 2 * m2 + half
                fcg = fcgs2[half]
                bk4 = ptp.tile([P, 4, P], BF16, tag="tpb")
                for jq in range(4):
                    nc.tensor.transpose(bk4[:, jq, :],
                                        fcg[:, jq * P:(jq + 1) * P], ident)
                nc.vector.tensor_tensor(xres[:, :, m * P:(m + 1) * P], bk4,
                                        xres[:, :, m * P:(m + 1) * P],
                                        ALU.add)

        # ---- Phase E: LN3 + mlp modulation -> n2T ---------------------------
        attp_cm.__exit__(None, None, None)
        abc_cm.__exit__(None, None, None)
        n2p_cm = tc.tile_pool(name="n2p", bufs=1)
        n2p = n2p_cm.__enter__()
        n2hi = n2p.tile([P, 8, TQ], FP8)
        n2lo = n2p.tile([P, 8, TQ], FP8)
        with tc.tile_pool(name="lnE", bufs=1) as lnE:
            l3 = [lnE.tile([P, D], BF16, name=f"l3_{s}") for s in range(4)]
            for s in range(4):
                mu, var = stats(xres[:, s, :], "act" if s % 2 else "dve")
                r3, b3 = rs_beta(mu, var, "2", 1.0, "r3")
                nc.gpsimd.tensor_scalar(l3[s], xres[:, s, :], r3, b3,
                                        ALU.mult, ALU.add)
            for m in range(8):
                bk4 = ptp.tile([P, 4, P], BF16, tag="tpb")
                for s in range(4):
                    nc.tensor.transpose(bk4[:, s, :],
                                        l3[s][:, m * P:(m + 1) * P], ident)
                bk4f = bk4.rearrange("p a b -> p (a b)")
                nc.vector.tensor_copy(n2hi[:, m, :], bk4f)
                nc.vector.tensor_tensor(n2lo[:, m, :], bk4f, n2hi[:, m, :],
                                        ALU.subtract)

        # ---- Phase F: FFN ---------------------------------------------------
        with tc.tile_pool(name="ffp", bufs=1) as ffp:
            ffahi = ffp.tile([P, 32, TQ], FP8)
            ffalo = ffp.tile([P, 32, TQ], FP8)
            for m8 in range(4):
                w1ch = fstr.tile([P, 8, 4, 2, P], FP8, tag="w1ch", bufs=2)
                nc.sync.dma_start(w1ch, dt["w1h"][:, m8 * 8:(m8 + 1) * 8])
                w1cl = fstr.tile([P, 8, 4, 2, P], FP8, tag="w1cl", bufs=2)
                nc.sync.dma_start(w1cl, dt["w1l"][:, m8 * 8:(m8 + 1) * 8])
                for m2 in range(4):
                    ps2 = psc.tile([P, 2, TQ], F32, tag="sc")
                    for half in range(2):
                        m_ = 2 * m2 + half
                        m = m8 * 8 + m_
                        first = True
                        for j in range(4):
                            xh = n2hi[:, 2 * j:2 * j + 2, :]
                            xl = n2lo[:, 2 * j:2 * j + 2, :]
                            for wgt, rhs in ((w1ch, xh), (w1ch, xl),
                                             (w1cl, xh)):
                                nc.tensor.matmul(
                                    ps2[:, half, :], wgt[:, m_, j], rhs,
                                    start=first,
                                    stop=(j == 3 and rhs is xh
                                          and wgt is w1cl),
                                    perf_mode=DR)
                                first = False
                        ffabf = lnio.tile([P, TQ], BF16, tag="ffabf",
                                          bufs=3)
                        nc.scalar.activation(ffabf, ps2[:, half, :],
                                             AF.Gelu_apprx_tanh,
                                             bias=fb1_s[:, m:m + 1],
                                             scale=WSI)
                        nc.gpsimd.tensor_copy(ffahi[:, m, :], ffabf)
                        nc.vector.tensor_tensor(ffalo[:, m, :], ffabf,
                                                ffahi[:, m, :], ALU.subtract)
            for m2 in range(4):
                ps2 = psc.tile([P, 2, TQ], F32, tag="sc")
                ffgs2 = []
                for half in range(2):
                    m = 2 * m2 + half
                    w2ch = fstr.tile([P, 16, 2, P], FP8, tag="w2ch", bufs=3)
                    nc.sync.dma_start(w2ch, dt["w2h"][:, m])
                    w2cl = fstr.tile([P, 16, 2, P], FP8, tag="w2cl", bufs=3)
                    nc.sync.dma_start(w2cl, dt["w2l"][:, m])
                    first = True
                    for j in range(16):
                        xh = ffahi[:, 2 * j:2 * j + 2, :]
                        xl = ffalo[:, 2 * j:2 * j + 2, :]
                        for wgt, rhs in ((w2ch, xh), (w2ch, xl), (w2cl, xh)):
                            nc.tensor.matmul(
                                ps2[:, half, :], wgt[:, j], rhs,
                                start=first,
                                stop=(j == 15 and rhs is xh and wgt is w2cl),
                                perf_mode=DR)
                            first = False
                    ps = ps2[:, half, :]
                    ffg = lnio.tile([P, TQ], BF16, tag="ffg", bufs=4)
                    gb = small.tile([P, 1], F32, tag="gb2")
                    nc.vector.tensor_tensor(gb, fb2_s[:, m:m + 1],
                                            g_mlp[:, m:m + 1], ALU.mult)
                    gs2 = small.tile([P, 1], F32, tag="gs2")
                    nc.vector.tensor_scalar_mul(gs2, g_mlp[:, m:m + 1], WSI)
                    nc.scalar.activation(ffg, ps, AF.Identity, bias=gb,
                                         scale=gs2)
                    ffgs2.append(ffg)
                for half in range(2):
                    m = 2 * m2 + half
                    ffg = ffgs2[half]
                    bk4 = ptp.tile([P, 4, P], BF16, tag="tpb")
                    for jq in range(4):
                        nc.tensor.transpose(bk4[:, jq, :],
                                            ffg[:, jq * P:(jq + 1) * P],
                                            ident)
                    nc.vector.tensor_tensor(xres[:, :, m * P:(m + 1) * P],
                                            bk4,
                                            xres[:, :, m * P:(m + 1) * P],
                                            ALU.add)

        n2p_cm.__exit__(None, None, None)
        for s in range(4):
            nc.sync.dma_start(out_ap[s * P:(s + 1) * P, :], xres[:, s, :])


# --------------------------- host side --------------------------------------
_NC_CACHE = None
_LAST_INMAPS = None


def _prep_conv_w(w):
    """(D, DK, KW) grouped conv -> [128, 8, 7, 2, 128] fp8 block-diag."""
    f8 = ml_dtypes.float8_e4m3
    wr = w.reshape(H, DK, DK, KW)            # [h, out, cin, tap]
    arr = np.zeros((P, 8, 7, 2, P), np.float32)
    for hp in range(8):
        for g in range(2):
            h = 2 * hp + g
            rs = slice(g * DK, (g + 1) * DK)
            for j in range(7):
                arr[rs, hp, j, 0, rs] = wr[h, :, :, 2 * j].T * WS
                if 2 * j + 1 < KW:
                    arr[rs, hp, j, 1, rs] = wr[h, :, :, 2 * j + 1].T * WS
    return arr.astype(f8)


def _prep_mm_w(wT, nm, nj):
    """wT [K, M] contraction-major -> [128, nm, nj, 2, 128] fp8."""
    f8 = ml_dtypes.float8_e4m3
    arr = wT.reshape(nj, 2, P, nm, P).transpose(2, 3, 0, 1, 4) * WS
    return np.ascontiguousarray(arr).astype(f8)


def kernel(**inputs):
    global _NC_CACHE, _LAST_INMAPS
    if _NC_CACHE is None:
        _NC_CACHE = build_nc()
    nc = _NC_CACHE

    f32 = np.float32
    f8 = ml_dtypes.float8_e4m3
    noisy = np.asarray(inputs["noisy_feats"], f32)
    clean = np.asarray(inputs["clean_feats"], f32)
    t = np.asarray(inputs["t"], f32)
    clean_len = np.asarray(inputs["clean_lengths"]).astype(np.int64)

    assert np.all(np.asarray(inputs["ln_noisy_g"], f32) == 1.0)
    assert np.all(np.asarray(inputs["ln_noisy_b"], f32) == 0.0)

    # AdaLayerNormZero on host (0.02% of FLOPs)
    st = t * (1.0 / (1.0 + np.exp(-t, dtype=f32)))
    emb = st @ np.asarray(inputs["ada_w"], f32).T + \
        np.asarray(inputs["ada_b"], f32)
    sh_msa, sc_msa, g_msa, sh_mlp, sc_mlp, g_mlp = np.split(emb, 6, axis=1)

    wql = _prep_conv_w(np.asarray(inputs["wq"], f32))
    wkl = _prep_conv_w(np.asarray(inputs["wk"], f32))
    wvl = _prep_conv_w(np.asarray(inputs["wv"], f32))
    fcw = _prep_mm_w(np.asarray(inputs["fc_w"], f32).T, 8, 4)

    def hilo(wT, nm, nj):
        ws = wT * WS
        hi = ws.astype(f8).astype(f32)
        lo = ws - hi
        return (_prep_mm_w(hi / WS, nm, nj), _prep_mm_w(lo / WS, nm, nj))

    w2h, w2l = hilo(np.asarray(inputs["ff_w2"], f32).T, 8, 16)
    ff_w1 = np.asarray(inputs["ff_w1"], f32)
    fb1_base = np.asarray(inputs["ff_b1"], f32)

    common = dict(
        clng=np.asarray(inputs["ln_clean_g"], f32).copy(),
        clnb=np.asarray(inputs["ln_clean_b"], f32).copy(),
        wq=wql, wk=wkl, wv=wvl,
        bq=np.asarray(inputs["bq"], f32).copy(),
        bk=np.asarray(inputs["bk"], f32).copy(),
        bv=np.asarray(inputs["bv"], f32).copy(),
        fcw=fcw, fcb=np.asarray(inputs["fc_b"], f32).copy(),
        w2h=w2h, w2l=w2l, fb2=np.asarray(inputs["ff_b2"], f32).copy(),
    )

    bfd = ml_dtypes.bfloat16
    in_maps = []
    for i in range(8):
        b, half = i // 2, i % 2
        t0 = half * TQ
        noisyH = np.zeros((NHW, D), f32)
        lo, hi = t0 - P, t0 + 640
        clo, chi = max(lo, 0), min(hi, T)
        noisyH[clo - lo:chi - lo] = noisy[b, clo:chi]
        # host-side LN stats (exact f32); applies happen on-chip
        lnst = np.zeros((36, P), f32)
        nmu = noisyH.mean(1)
        nvar = noisyH.var(1)
        r2v = 1.0 / np.sqrt(nvar * (1 + EPS2) + EPS1 * EPS2)
        r1v = 1.0 / np.sqrt(nvar + EPS1)
        lnst[0:6] = r2v.reshape(6, P)
        lnst[6:12] = (-nmu * r2v).reshape(6, P)
        lnst[12:16] = r1v.reshape(6, P)[1:5]
        lnst[16:20] = (-nmu * r1v).reshape(6, P)[1:5]
        cmu = clean[b].mean(1)
        cvar = clean[b].var(1)
        rcv = 1.0 / np.sqrt(cvar + EPS1)
        lnst[20:28] = rcv.reshape(8, P)
        lnst[28:36] = (-cmu * rcv).reshape(8, P)
        mvec = (np.arange(T) < clean_len[b]).astype(f32)
        mk = np.broadcast_to(
            mvec.reshape(8, P).T[:, :, None, None], (P, 8, 2, DK)).astype(f8)
        eL = 0.0 if half == 0 else 1.0
        eR = 1.0 if half == 0 else 0.0
        mod = np.stack([
            sh_msa[b], 1.0 + sc_msa[b], g_msa[b],
            sh_mlp[b], 1.0 + sc_mlp[b], g_mlp[b],
            sh_msa[b] * eL, (1.0 + sc_msa[b]) * eL,
            sh_msa[b] * eR, (1.0 + sc_msa[b]) * eR,
        ]).astype(f32)
        w1b = ff_w1 * (1.0 + sc_mlp[b])[None, :]
        w1h, w1l = hilo(w1b.T, 32, 4)
        fb1 = fb1_base + ff_w1 @ sh_mlp[b]
        m = dict(common)
        m.update(noisyH=noisyH.astype(bfd), clean=clean[b].astype(bfd),
                 mod=mod, mk=np.ascontiguousarray(mk), lnst=lnst,
                 w1h=w1h, w1l=w1l, fb1=fb1)
        in_maps.append(m)

    _LAST_INMAPS = in_maps
    res = run_bass_kernel_spmd(nc, in_maps, core_ids=list(range(8)))
    out = np.empty((B, T, D), f32)
    for i in range(8):
        b, half = i // 2, i % 2
        out[b, half * TQ:(half + 1) * TQ] = res.results[i]["out"]
    return out


if __name__ == "__main__":
    build_nc()
    print("build ok")


# revision 6
# speedup vs baseline: 1.2064x; 1.0095x over previous
# Trainium2 Bass kernel for nn_CrossAttention_noise (B=4, T1=T2=1024, D=1024,
# H=16, DK=64, KW=13, FF=4096), SPMD over 8 NeuronCores.
#
# Sharding: core i handles batch b=i//2 and query-token half t0=(i%2)*512.
# All heavy matmuls run in fp8e4 with DoubleRow perf mode (2 K-tiles per
# pass):
#   - grouped convs: 2 heads per matmul via block-diagonal weights; the two
#     K-tiles are the shift-0/shift-1 planes of the transposed input (the
#     shift-1 plane is a SBUF->SBUF DMA copy), so one DR matmul covers 2 taps
#     x 128 channels; 7 matmuls cover the padded 14 taps.
#   - scores: lhsT/rhs use a stride-0 "pair broadcast" so the DR pass
#     computes 2*k^T q; the exp compensates with scale 1/16.
#   - PV (token-major out), fc, FFN: K-tile pairs are adjacent 128-ch blocks.
# The key-padding mask is folded into v65 (value rows and the ones-column
# multiplied by 0/1), so exp needs no per-chunk bias and one Activation op
# covers two score chunks (a 2-bank PSUM tile).
import numpy as np
import ml_dtypes
from contextlib import ExitStack

import concourse.bass as bass
import concourse.mybir as mybir
import concourse.tile as tile
from concourse import bacc
from concourse.bass_utils import run_bass_kernel_spmd
from concourse.masks import make_identity

BF16 = mybir.dt.bfloat16
F32 = mybir.dt.float32
FP8 = mybir.dt.float8e4
DR = mybir.MatmulPerfMode.DoubleRow
AF = mybir.ActivationFunctionType
ALU = mybir.AluOpType
AX = mybir.AxisListType

B, T, D, H, DK, KW, FF = 4, 1024, 1024, 16, 64, 13, 4096
TQ = 512           # query tokens per core
P = 128
NHW = 768          # noisy halo window rows (zero-padded on host)
NTW = 528          # nt2 plane width (524 used + 4 pad)
CTW = 1040         # ct2 plane width (1036 used + 4 pad)
EPS1, EPS2 = 1e-5, 1e-6
WS = 32.0          # host-side weight scale (power of two)
WSI = 1.0 / WS


def pair_b(ap2d):
    """[p, N] -> [p, 2, N] stride-0 plane broadcast (for double-q scores)."""
    p, n = ap2d.shape
    return ap2d.unsqueeze(1).broadcast_to((p, 2, n))


def build_nc():
    nc = bacc.Bacc("TRN2", target_bir_lowering=False, debug=False,
                   num_devices=8)
    dt = {}

    def din(name, shape, dtype):
        dt[name] = nc.dram_tensor(name, list(shape), dtype,
                                  kind="ExternalInput").ap()

    din("noisyH", (NHW, D), BF16)      # rows [t0-128, t0+640), zero padded
    din("clean", (T, D), BF16)
    din("lnst", (36, P), F32)          # host LN stats: r2[6],b2[6],r1[4],
    #                                     b1[4], rc[8],bc[8]
    din("mod", (10, D), F32)           # sh_m,1+sc_m,g_m,sh_f,1+sc_f,g_f,
    #                                     shL,(1+sc)L,shR,(1+sc)R (edge-masked)
    din("clng", (D,), F32)
    din("clnb", (D,), F32)
    din("wq", (P, 8, 7, 2, P), FP8)    # [cin][pair][tap-pair j][plane][cout]
    din("wk", (P, 8, 7, 2, P), FP8)
    din("wv", (P, 8, 7, 2, P), FP8)
    din("bq", (D,), F32)
    din("bk", (D,), F32)
    din("bv", (D,), F32)
    din("mk", (P, 8, 2, DK), FP8)      # key mask replicated (part,chunk,h2,dk)
    din("fcw", (P, 8, 4, 2, P), FP8)   # [ic][m][j][plane][oc]
    din("fcb", (D,), F32)
    din("w1h", (P, 32, 4, 2, P), FP8)  # [ic][m][j][plane][oc] hi level
    din("w1l", (P, 32, 4, 2, P), FP8)  # lo level (residual)
    din("fb1", (FF,), F32)
    din("w2h", (P, 8, 16, 2, P), FP8)
    din("w2l", (P, 8, 16, 2, P), FP8)
    din("fb2", (D,), F32)
    out_ap = nc.dram_tensor("out", [TQ, D], F32, kind="ExternalOutput").ap()

    with tile.TileContext(nc) as tc:
        _emit(tc, dt, out_ap)
    nc.compile()
    return nc


def _emit(tc, dt, out_ap):
    nc = tc.nc
    with ExitStack() as ctx:
        const = ctx.enter_context(tc.tile_pool(name="const", bufs=1))
        small = ctx.enter_context(tc.tile_pool(name="small", bufs=4))
        lnio = ctx.enter_context(tc.tile_pool(name="lnio", bufs=4))
        big = ctx.enter_context(tc.tile_pool(name="bigsb", bufs=1))
        fstr = ctx.enter_context(tc.tile_pool(name="fstr", bufs=2))
        pcv = ctx.enter_context(tc.tile_pool(name="pcv", bufs=1, space="PSUM"))
        psc = ctx.enter_context(tc.tile_pool(name="psc", bufs=2, space="PSUM"))
        ptp = ctx.enter_context(tc.tile_pool(name="ptp", bufs=2, space="PSUM"))
        ppv = ctx.enter_context(tc.tile_pool(name="ppv", bufs=1, space="PSUM"))

        ident = const.tile([P, P], BF16)
        make_identity(nc, ident)
        e_1 = const.tile([P, 1], F32)
        nc.vector.memset(e_1, EPS1)
        e_12 = const.tile([P, 1], F32)
        nc.vector.memset(e_12, EPS1 * EPS2)
        e_2 = const.tile([P, 1], F32)
        nc.vector.memset(e_2, EPS2)
        EPS_T = {"1": e_1, "12": e_12, "2": e_2}

        def chanvec(name, w=8):
            t = const.tile([P, w], F32, tag=f"cv_{name}")
            nc.sync.dma_start(t, dt[name].rearrange("(m p) -> p m", p=P))
            return t

        bq_s, bk_s, bv_s = chanvec("bq"), chanvec("bk"), chanvec("bv")
        fcb_s, fb2_s = chanvec("fcb"), chanvec("fb2")
        clng_s, clnb_s = chanvec("clng"), chanvec("clnb")
        fb1_s = chanvec("fb1", 32)
        mod_s = const.tile([P, 10, 8], F32)
        for s in range(10):
            nc.sync.dma_start(mod_s[:, s, :],
                              dt["mod"][s].rearrange("(m p) -> p m", p=P))
        sh_msa, sc_msa, g_msa = mod_s[:, 0, :], mod_s[:, 1, :], mod_s[:, 2, :]
        sh_mlp, sc_mlp, g_mlp = mod_s[:, 3, :], mod_s[:, 4, :], mod_s[:, 5, :]
        shL, scL = mod_s[:, 6, :], mod_s[:, 7, :]
        shR, scR = mod_s[:, 8, :], mod_s[:, 9, :]
        mk_s = const.tile([P, 8, 2, DK], FP8)
        nc.sync.dma_start(mk_s, dt["mk"])
        lnst_s = const.tile([P, 36], F32)
        nc.sync.dma_start(lnst_s, dt["lnst"].rearrange("n p -> p n"))

        def stats(x, eng_sq):
            """(mu, var) [p,1] f32 for rows of x [p, D]."""
            s = small.tile([P, 1], F32, tag="st_s")
            nc.vector.reduce_sum(s, x, axis=AX.X)
            sq = small.tile([P, 1], F32, tag="st_sq")
            scr = small.tile([P, D], BF16, tag="st_scr", bufs=2)
            nc.scalar.activation(scr, x, AF.Square, accum_out=sq)
            mu = small.tile([P, 1], F32, tag="st_mu")
            nc.vector.tensor_scalar_mul(mu, s, 1.0 / D)
            msq = small.tile([P, 1], F32, tag="st_msq")
            nc.vector.tensor_tensor(msq, mu, mu, ALU.mult)
            var = small.tile([P, 1], F32, tag="st_var")
            nc.vector.tensor_scalar(var, sq, 1.0 / D, msq,
                                    ALU.mult, ALU.subtract)
            return mu, var

        def rs_beta(mu, var, eps_key, scale, tag):
            """(r, beta) with r = 1/sqrt(var*scale+eps), beta = -mu*r."""
            st = small.tile([P, 1], F32, tag=tag + "_s")
            nc.scalar.activation(st, var, AF.Sqrt, bias=EPS_T[eps_key],
                                 scale=scale)
            rt = small.tile([P, 1], F32, tag=tag + "_r")
            nc.vector.reciprocal(rt, st)
            bt = small.tile([P, 1], F32, tag=tag + "_b")
            nc.vector.tensor_tensor(bt, mu, rt, ALU.mult)
            nc.vector.tensor_scalar_mul(bt, bt, -1.0)
            return rt, bt

        xres = big.tile([P, 4, D], F32)        # y1 rows [t0, t0+512); then x
        abc_cm = tc.tile_pool(name="abc", bufs=1)
        abc = abc_cm.__enter__()
        nt2 = [abc.tile([P, 2, NTW], FP8, name=f"nt2_{m}") for m in range(8)]
        ct2 = [abc.tile([P, 2, CTW], FP8, name=f"ct2_{m}") for m in range(8)]
        qT = [abc.tile([P, TQ], FP8, name=f"qT_{m}") for m in range(8)]


        # one-time pad zeroing (tiles are persistent)
        for m in range(8):
            nc.gpsimd.memset(nt2[m][:, 0, 524:528], 0.0)
            nc.gpsimd.memset(nt2[m][:, 1, 523:528], 0.0)
            nc.gpsimd.memset(ct2[m][:, 0, 0:6], 0.0)
            nc.gpsimd.memset(ct2[m][:, 0, 1030:1040], 0.0)
            nc.gpsimd.memset(ct2[m][:, 1, 1029:1040], 0.0)

        # ---- Phase A: noisy stats -> y1 (xres) + y2 -> nt2 + Q convs --------
        with tc.tile_pool(name="lnA", bufs=1) as lnA:
            y2 = [lnA.tile([P, D], BF16, name=f"y2_{r}") for r in range(6)]
            for r in range(6):
                xt = lnio.tile([P, D], BF16, tag="ln_in", bufs=3)
                nc.sync.dma_start(xt, dt["noisyH"][r * P:(r + 1) * P, :])
                nc.vector.tensor_scalar(y2[r], xt, lnst_s[:, r:r + 1],
                                        lnst_s[:, 6 + r:7 + r],
                                        ALU.mult, ALU.add)
                if 1 <= r <= 4:
                    nc.gpsimd.tensor_scalar(xres[:, r - 1, :], xt,
                                            lnst_s[:, 11 + r:12 + r],
                                            lnst_s[:, 15 + r:16 + r],
                                            ALU.mult, ALU.add)
            for m in range(8):
                bA = ptp.tile([P, 4, P], BF16, tag="tpb")
                bB = ptp.tile([P, 4, P], BF16, tag="tpb")
                for r in range(6):
                    dst = bA[:, r, :] if r < 4 else bB[:, r - 4, :]
                    nc.tensor.transpose(dst, y2[r][:, m * P:(m + 1) * P],
                                        ident)
                mm, ms = sc_msa[:, m:m + 1], sh_msa[:, m:m + 1]
                nc.vector.tensor_scalar(nt2[m][:, 0, 0:6], bA[:, 0, 122:128],
                                        scL[:, m:m + 1], shL[:, m:m + 1],
                                        ALU.mult, ALU.add)
                nc.vector.tensor_scalar(
                    nt2[m][:, 0, 6:390],
                    bA.rearrange("p a b -> p (a b)")[:, 128:512],
                    mm, ms, ALU.mult, ALU.add)
                nc.vector.tensor_scalar(nt2[m][:, 0, 390:518],
                                        bB[:, 0, :], mm, ms,
                                        ALU.mult, ALU.add)
                nc.vector.tensor_scalar(nt2[m][:, 0, 518:524], bB[:, 1, 0:6],
                                        scR[:, m:m + 1], shR[:, m:m + 1],
                                        ALU.mult, ALU.add)
                nc.sync.dma_start(nt2[m][:, 1, 0:523], nt2[m][:, 0, 1:524])
        for hp in range(8):
            if hp % 4 == 0:
                wqs = abc.tile([P, 4, 7, 2, P], FP8, tag="wqs", bufs=2)
                nc.sync.dma_start(wqs, dt["wq"][:, hp:hp + 4])
            ps = pcv.tile([P, TQ], F32, tag="conv")
            for j in range(7):
                nc.tensor.matmul(ps, wqs[:, hp % 4, j],
                                 nt2[hp][:, :, 2 * j:2 * j + TQ],
                                 start=(j == 0), stop=(j == 6), perf_mode=DR)
            nc.vector.tensor_scalar(qT[hp], ps, WSI, bq_s[:, hp:hp + 1],
                                    ALU.mult, ALU.add)

        # ---- Phase B: clean stats -> y -> ct2 -------------------------------
        with tc.tile_pool(name="lnB", bufs=1) as lnB:
            yc = [lnB.tile([P, D], BF16, name=f"yc_{r}") for r in range(8)]
            for r in range(8):
                xt = lnio.tile([P, D], BF16, tag="ln_in", bufs=3)
                nc.sync.dma_start(xt, dt["clean"][r * P:(r + 1) * P, :])
                nc.vector.tensor_scalar(yc[r], xt, lnst_s[:, 20 + r:21 + r],
                                        lnst_s[:, 28 + r:29 + r],
                                        ALU.mult, ALU.add)
            for m in range(8):
                gg, bb = clng_s[:, m:m + 1], clnb_s[:, m:m + 1]
                for half in range(2):
                    bk4 = ptp.tile([P, 4, P], BF16, tag="tpb")
                    for r4 in range(4):
                        nc.tensor.transpose(
                            bk4[:, r4, :],
                            yc[half * 4 + r4][:, m * P:(m + 1) * P], ident)
                    nc.vector.tensor_scalar(
                        ct2[m][:, 0, 6 + half * 512:518 + half * 512],
                        bk4.rearrange("p a b -> p (a b)"), gg, bb,
                        ALU.mult, ALU.add)
                nc.sync.dma_start(ct2[m][:, 1, 0:1039], ct2[m][:, 0, 1:1040])

        # ---- Phase C: per-pair K/V conv + attention -------------------------
        attp_cm = tc.tile_pool(name="attp", bufs=1)
        attp = attp_cm.__enter__()
        attnT = attp.tile([P, 8, TQ], FP8)
        attnQ = attp.tile([P, 4, H, DK], BF16)
        with tc.tile_pool(name="hpool", bufs=2) as hpool:
            for hp in range(8):
                if hp % 4 == 0:
                    wks = abc.tile([P, 4, 7, 2, P], FP8, tag="wks", bufs=2)
                    nc.sync.dma_start(wks, dt["wk"][:, hp:hp + 4])
                    wvs = abc.tile([P, 4, 7, 2, P], FP8, tag="wvs", bufs=2)
                    nc.sync.dma_start(wvs, dt["wv"][:, hp:hp + 4])
                kT = hpool.tile([P, 2, TQ], FP8, tag="kT")
                for c in range(2):
                    ps = pcv.tile([P, TQ], F32, tag="conv")
                    for j in range(7):
                        nc.tensor.matmul(
                            ps, wks[:, hp % 4, j],
                            ct2[hp][:, :, 2 * j + c * TQ:2 * j + (c + 1) * TQ],
                            start=(j == 0), stop=(j == 6), perf_mode=DR)
                    nc.vector.tensor_scalar(kT[:, c, :], ps, WSI,
                                            bk_s[:, hp:hp + 1],
                                            ALU.mult, ALU.add)
                vT = hpool.tile([P, 2, TQ], BF16, tag="vT")
                for c in range(2):
                    ps = pcv.tile([P, TQ], F32, tag="conv")
                    for j in range(7):
                        nc.tensor.matmul(
                            ps, wvs[:, hp % 4, j],
                            ct2[hp][:, :, 2 * j + c * TQ:2 * j + (c + 1) * TQ],
                            start=(j == 0), stop=(j == 6), perf_mode=DR)
                    nc.vector.tensor_scalar(vT[:, c, :], ps, WSI,
                                            bv_s[:, hp:hp + 1],
                                            ALU.mult, ALU.add)
                # v65: token-major masked values + masked ones column
                v65 = hpool.tile([P, 8, 2, 80], FP8, tag="v65")
                nc.vector.tensor_copy(v65[:, :, :, 64], mk_s[:, :, :, 0])
                vTf = vT.rearrange("p a b -> p (a b)")
                for half in range(2):
                    bk4 = ptp.tile([P, 4, P], BF16, tag="tpb")
                    for c4 in range(4):
                        cc = half * 4 + c4
                        nc.tensor.transpose(bk4[:, c4, :],
                                            vTf[:, cc * P:(cc + 1) * P],
                                            ident)
                    nc.vector.tensor_tensor(
                        v65[:, half * 4:half * 4 + 4, :, 0:64],
                        bk4.rearrange("p a (h d) -> p a h d", h=2),
                        mk_s[:, half * 4:half * 4 + 4, :, :], ALU.mult)
                kTf = kT.rearrange("p a b -> p (a b)")
                for hh in range(2):
                    h = 2 * hp + hh
                    rows = slice(hh * DK, (hh + 1) * DK)
                    pT = hpool.tile([P, 8, TQ], FP8, tag="pT")
                    for c2 in range(4):
                        ps = psc.tile([P, 2, TQ], F32, tag="sc")
                        for c in range(2):
                            cc = 2 * c2 + c
                            nc.tensor.matmul(
                                ps[:, c, :],
                                pair_b(kTf[rows, cc * P:(cc + 1) * P]),
                                pair_b(qT[hp][rows, :]),
                                start=True, stop=True, perf_mode=DR)
                        nc.scalar.activation(
                            pT[:, 2 * c2:2 * c2 + 2, :].rearrange(
                                "p a b -> p (a b)"),
                            ps.rearrange("p a b -> p (a b)"), AF.Exp,
                            scale=0.0625)
                    pv = ppv.tile([P, 4, 65], F32, tag="pv")
                    for jq in range(4):
                        for c2 in range(4):
                            nc.tensor.matmul(
                                pv[:, jq, :],
                                pT[:, 2 * c2:2 * c2 + 2, jq * P:(jq + 1) * P],
                                v65[:, 2 * c2:2 * c2 + 2, hh, 0:65],
                                start=(c2 == 0), stop=(c2 == 3), perf_mode=DR)
                    rec = small.tile([P, 4], F32, tag="rec")
                    nc.vector.reciprocal(rec, pv[:, :, 64])
                    for jq in range(4):
                        nc.vector.tensor_scalar_mul(attnQ[:, jq, h, :],
                                                    pv[:, jq, 0:64],
                                                    rec[:, jq:jq + 1])

        abc_cm.__exit__(None, None, None)

        # ---- Phase D: attnT transposes + fc + residual ----------------------
        aqf = attnQ.rearrange("p a h d -> p (a h d)")
        for hp in range(8):
            bk4 = ptp.tile([P, 4, P], BF16, tag="tpb")
            for jq in range(4):
                nc.tensor.transpose(
                    bk4[:, jq, :],
                    aqf[:, jq * 1024 + hp * P:jq * 1024 + (hp + 1) * P],
                    ident)
            nc.vector.tensor_copy(attnT[:, hp, :],
                                  bk4.rearrange("p a b -> p (a b)"))
        fcws = attp.tile([P, 8, 4, 2, P], FP8, name="fcws")
        nc.sync.dma_start(fcws, dt["fcw"])
        for m2 in range(4):
            ps2 = psc.tile([P, 2, TQ], F32, tag="sc")
            for half in range(2):
                for j in range(4):
                    nc.tensor.matmul(ps2[:, half, :],
                                     fcws[:, 2 * m2 + half, j],
                                     attnT[:, 2 * j:2 * j + 2, :],
                                     start=(j == 0), stop=(j == 3),
                                     perf_mode=DR)
          # per-m evacs below read ps2 planes
            fcgs2 = []
            for half in range(2):
                m = 2 * m2 + half
                ps = ps2[:, half, :]
                fcg = lnio.tile([P, TQ], BF16, tag="fcg", bufs=4)
                gb = small.tile([P, 1], F32, tag="gb")
                nc.vector.tensor_tensor(gb, fcb_s[:, m:m + 1],
                                        g_msa[:, m:m + 1], ALU.mult)
                gsc = small.tile([P, 1], F32, tag="gsc")
                nc.vector.tensor_scalar_mul(gsc, g_msa[:, m:m + 1], WSI)
                nc.scalar.activation(fcg, ps, AF.Identity, bias=gb, scale=gsc)
                fcgs2.append(fcg)
            for half in range(2):
                m = 2 * m2 + half
                fcg = fcgs2[half]
                bk4 = ptp.tile([P, 4, P], BF16, tag="tpb")
                for jq in range(4):
                    nc.tensor.transpose(bk4[:, jq, :],
                                        fcg[:, jq * P:(jq + 1) * P], ident)
                nc.vector.tensor_tensor(xres[:, :, m * P:(m + 1) * P], bk4,
                                        xres[:, :, m * P:(m + 1) * P],
                                        ALU.add)

        # ---- Phase E: LN3 + mlp modulation -> n2T ---------------------------
        attp_cm.__exit__(None, None, None)
        abc_cm.__exit__(None, None, None)
        n2p_cm = tc.tile_pool(name="n2p", bufs=1)
        n2p = n2p_cm.__enter__()
        n2hi = n2p.tile([P, 8, TQ], FP8)
        n2lo = n2p.tile([P, 8, TQ], FP8)
        with tc.tile_pool(name="lnE", bufs=1) as lnE:
            l3 = [lnE.tile([P, D], BF16, name=f"l3_{s}") for s in range(4)]
            for s in range(4):
                mu, var = stats(xres[:, s, :], "act" if s % 2 else "dve")
                r3, b3 = rs_beta(mu, var, "2", 1.0, "r3")
                nc.vector.tensor_scalar(l3[s], xres[:, s, :], r3, b3,
                                        ALU.mult, ALU.add)
            for m in range(8):
                bk4 = ptp.tile([P, 4, P], BF16, tag="tpb")
                for s in range(4):
                    nc.tensor.transpose(bk4[:, s, :],
                                        l3[s][:, m * P:(m + 1) * P], ident)
                bk4f = bk4.rearrange("p a b -> p (a b)")
                nc.vector.tensor_copy(n2hi[:, m, :], bk4f)
                nc.vector.tensor_tensor(n2lo[:, m, :], bk4f, n2hi[:, m, :],
                                        ALU.subtract)

        # ---- Phase F: FFN ---------------------------------------------------
        with tc.tile_pool(name="ffp", bufs=1) as ffp:
            ffahi = ffp.tile([P, 32, TQ], FP8)
            ffalo = ffp.tile([P, 32, TQ], FP8)
            for m8 in range(4):
                w1ch = fstr.tile([P, 8, 4, 2, P], FP8, tag="w1ch", bufs=2)
                nc.sync.dma_start(w1ch, dt["w1h"][:, m8 * 8:(m8 + 1) * 8])
                w1cl = fstr.tile([P, 8, 4, 2, P], FP8, tag="w1cl", bufs=2)
                nc.sync.dma_start(w1cl, dt["w1l"][:, m8 * 8:(m8 + 1) * 8])
                for m2 in range(4):
                    ps2 = psc.tile([P, 2, TQ], F32, tag="sc")
                    for half in range(2):
                        m_ = 2 * m2 + half
                        m = m8 * 8 + m_
                        first = True
                        for j in range(4):
                            xh = n2hi[:, 2 * j:2 * j + 2, :]
                            xl = n2lo[:, 2 * j:2 * j + 2, :]
                            for wgt, rhs in ((w1ch, xh), (w1ch, xl),
                                             (w1cl, xh)):
                                nc.tensor.matmul(
                                    ps2[:, half, :], wgt[:, m_, j], rhs,
                                    start=first,
                                    stop=(j == 3 and rhs is xh
                                          and wgt is w1cl),
                                    perf_mode=DR)
                                first = False
                        ffabf = lnio.tile([P, TQ], BF16, tag="ffabf",
                                          bufs=3)
                        nc.scalar.activation(ffabf, ps2[:, half, :],
                                             AF.Gelu_apprx_tanh,
                                             bias=fb1_s[:, m:m + 1],
                                             scale=WSI)
                        nc.gpsimd.tensor_copy(ffahi[:, m, :], ffabf)
                        nc.vector.tensor_tensor(ffalo[:, m, :], ffabf,
                                                ffahi[:, m, :], ALU.subtract)
            for m2 in range(4):
                ps2 = psc.tile([P, 2, TQ], F32, tag="sc")
                ffgs2 = []
                for half in range(2):
                    m = 2 * m2 + half
                    w2ch = fstr.tile([P, 16, 2, P], FP8, tag="w2ch", bufs=3)
                    nc.sync.dma_start(w2ch, dt["w2h"][:, m])
                    w2cl = fstr.tile([P, 16, 2, P], FP8, tag="w2cl", bufs=3)
                    nc.sync.dma_start(w2cl, dt["w2l"][:, m])
                    first = True
                    for j in range(16):
                        xh = ffahi[:, 2 * j:2 * j + 2, :]
                        xl = ffalo[:, 2 * j:2 * j + 2, :]
                        for wgt, rhs in ((w2ch, xh), (w2ch, xl), (w2cl, xh)):
                            nc.tensor.matmul(
                                ps2[:, half, :], wgt[:, j], rhs,
                                start=first,
                                stop=(j == 15 and rhs is xh and wgt is w2cl),
                                perf_mode=DR)
                            first = False
                    ps = ps2[:, half, :]
                    ffg = lnio.tile([P, TQ], BF16, tag="ffg", bufs=4)
                    gb = small.tile([P, 1], F32, tag="gb2")
                    nc.vector.tensor_tensor(gb, fb2_s[:, m:m + 1],
                                            g_mlp[:, m:m + 1], ALU.mult)
                    gs2 = small.tile([P, 1], F32, tag="gs2")
                    nc.vector.tensor_scalar_mul(gs2, g_mlp[:, m:m + 1], WSI)
                    nc.scalar.activation(ffg, ps, AF.Identity, bias=gb,
                                         scale=gs2)
                    ffgs2.append(ffg)
                for half in range(2):
                    m = 2 * m2 + half
                    ffg = ffgs2[half]
                    bk4 = ptp.tile([P, 4, P], BF16, tag="tpb")
                    for jq in range(4):
                        nc.tensor.transpose(bk4[:, jq, :],
                                            ffg[:, jq * P:(jq + 1) * P],
                                            ident)
                    nc.vector.tensor_tensor(xres[:, :, m * P:(m + 1) * P],
                                            bk4,
                                            xres[:, :, m * P:(m + 1) * P],
                                            ALU.add)

        n2p_cm.__exit__(None, None, None)
        for s in range(4):
            nc.sync.dma_start(out_ap[s * P:(s + 1) * P, :], xres[:, s, :])


# --------------------------- host side --------------------------------------
_NC_CACHE = None
_LAST_INMAPS = None


def _prep_conv_w(w):
    """(D, DK, KW) grouped conv -> [128, 8, 7, 2, 128] fp8 block-diag."""
    f8 = ml_dtypes.float8_e4m3
    wr = w.reshape(H, DK, DK, KW)            # [h, out, cin, tap]
    arr = np.zeros((P, 8, 7, 2, P), np.float32)
    for hp in range(8):
        for g in range(2):
            h = 2 * hp + g
            rs = slice(g * DK, (g + 1) * DK)
            for j in range(7):
                arr[rs, hp, j, 0, rs] = wr[h, :, :, 2 * j].T * WS
                if 2 * j + 1 < KW:
                    arr[rs, hp, j, 1, rs] = wr[h, :, :, 2 * j + 1].T * WS
    return arr.astype(f8)


def _prep_mm_w(wT, nm, nj):
    """wT [K, M] contraction-major -> [128, nm, nj, 2, 128] fp8."""
    f8 = ml_dtypes.float8_e4m3
    arr = wT.reshape(nj, 2, P, nm, P).transpose(2, 3, 0, 1, 4) * WS
    return np.ascontiguousarray(arr).astype(f8)


def kernel(**inputs):
    global _NC_CACHE, _LAST_INMAPS
    if _NC_CACHE is None:
        _NC_CACHE = build_nc()
    nc = _NC_CACHE

    f32 = np.float32
    f8 = ml_dtypes.float8_e4m3
    noisy = np.asarray(inputs["noisy_feats"], f32)
    clean = np.asarray(inputs["clean_feats"], f32)
    t = np.asarray(inputs["t"], f32)
    clean_len = np.asarray(inputs["clean_lengths"]).astype(np.int64)

    assert np.all(np.asarray(inputs["ln_noisy_g"], f32) == 1.0)
    assert np.all(np.asarray(inputs["ln_noisy_b"], f32) == 0.0)

    # AdaLayerNormZero on host (0.02% of FLOPs)
    st = t * (1.0 / (1.0 + np.exp(-t, dtype=f32)))
    emb = st @ np.asarray(inputs["ada_w"], f32).T + \
        np.asarray(inputs["ada_b"], f32)
    sh_msa, sc_msa, g_msa, sh_mlp, sc_mlp, g_mlp = np.split(emb, 6, axis=1)

    wql = _prep_conv_w(np.asarray(inputs["wq"], f32))
    wkl = _prep_conv_w(np.asarray(inputs["wk"], f32))
    wvl = _prep_conv_w(np.asarray(inputs["wv"], f32))
    fcw = _prep_mm_w(np.asarray(inputs["fc_w"], f32).T, 8, 4)

    def hilo(wT, nm, nj):
        ws = wT * WS
        hi = ws.astype(f8).astype(f32)
        lo = ws - hi
        return (_prep_mm_w(hi / WS, nm, nj), _prep_mm_w(lo / WS, nm, nj))

    w2h, w2l = hilo(np.asarray(inputs["ff_w2"], f32).T, 8, 16)
    ff_w1 = np.asarray(inputs["ff_w1"], f32)
    fb1_base = np.asarray(inputs["ff_b1"], f32)

    common = dict(
        clng=np.asarray(inputs["ln_clean_g"], f32).copy(),
        clnb=np.asarray(inputs["ln_clean_b"], f32).copy(),
        wq=wql, wk=wkl, wv=wvl,
        bq=np.asarray(inputs["bq"], f32).copy(),
        bk=np.asarray(inputs["bk"], f32).copy(),
        bv=np.asarray(inputs["bv"], f32).copy(),
        fcw=fcw, fcb=np.asarray(inputs["fc_b"], f32).copy(),
        w2h=w2h, w2l=w2l, fb2=np.asarray(inputs["ff_b2"], f32).copy(),
    )

    bfd = ml_dtypes.bfloat16
    in_maps = []
    for i in range(8):
        b, half = i // 2, i % 2
        t0 = half * TQ
        noisyH = np.zeros((NHW, D), f32)
        lo, hi = t0 - P, t0 + 640
        clo, chi = max(lo, 0), min(hi, T)
        noisyH[clo - lo:chi - lo] = noisy[b, clo:chi]
        # host-side LN stats (exact f32); applies happen on-chip
        lnst = np.zeros((36, P), f32)
        nmu = noisyH.mean(1)
        nvar = noisyH.var(1)
        r2v = 1.0 / np.sqrt(nvar * (1 + EPS2) + EPS1 * EPS2)
        r1v = 1.0 / np.sqrt(nvar + EPS1)
        lnst[0:6] = r2v.reshape(6, P)
        lnst[6:12] = (-nmu * r2v).reshape(6, P)
        lnst[12:16] = r1v.reshape(6, P)[1:5]
        lnst[16:20] = (-nmu * r1v).reshape(6, P)[1:5]
        cmu = clean[b].mean(1)
        cvar = clean[b].var(1)
        rcv = 1.0 / np.sqrt(cvar + EPS1)
        lnst[20:28] = rcv.reshape(8, P)
        lnst[28:36] = (-cmu * rcv).reshape(8, P)
        mvec = (np.arange(T) < clean_len[b]).astype(f32)
        mk = np.broadcast_to(
            mvec.reshape(8, P).T[:, :, None, None], (P, 8, 2, DK)).astype(f8)
        eL = 0.0 if half == 0 else 1.0
        eR = 1.0 if half == 0 else 0.0
        mod = np.stack([
            sh_msa[b], 1.0 + sc_msa[b], g_msa[b],
            sh_mlp[b], 1.0 + sc_mlp[b], g_mlp[b],
            sh_msa[b] * eL, (1.0 + sc_msa[b]) * eL,
            sh_msa[b] * eR, (1.0 + sc_msa[b]) * eR,
        ]).astype(f32)
        w1b = ff_w1 * (1.0 + sc_mlp[b])[None, :]
        w1h, w1l = hilo(w1b.T, 32, 4)
        fb1 = fb1_base + ff_w1 @ sh_mlp[b]
        m = dict(common)
        m.update(noisyH=noisyH.astype(bfd), clean=clean[b].astype(bfd),
                 mod=mod, mk=np.ascontiguousarray(mk), lnst=lnst,
                 w1h=w1h, w1l=w1l, fb1=fb1)
        in_maps.append(m)

    _LAST_INMAPS = in_maps
    res = run_bass_kernel_spmd(nc, in_maps, core_ids=list(range(8)))
    out = np.empty((B, T, D), f32)
    for i in range(8):
        b, half = i // 2, i % 2
        out[b, half * TQ:(half + 1) * TQ] = res.results[i]["out"]
    return out


if __name__ == "__main__":
    build_nc()
    print("build ok")


# revision 7
# speedup vs baseline: 1.2080x; 1.0013x over previous
# Trainium2 Bass kernel for nn_CrossAttention_noise (B=4, T1=T2=1024, D=1024,
# H=16, DK=64, KW=13, FF=4096), SPMD over 8 NeuronCores.
#
# Sharding: core i handles batch b=i//2 and query-token half t0=(i%2)*512.
# All heavy matmuls run in fp8e4 with DoubleRow perf mode (2 K-tiles per
# pass):
#   - grouped convs: 2 heads per matmul via block-diagonal weights; the two
#     K-tiles are the shift-0/shift-1 planes of the transposed input (the
#     shift-1 plane is a SBUF->SBUF DMA copy), so one DR matmul covers 2 taps
#     x 128 channels; 7 matmuls cover the padded 14 taps.
#   - scores: lhsT/rhs use a stride-0 "pair broadcast" so the DR pass
#     computes 2*k^T q; the exp compensates with scale 1/16.
#   - PV (token-major out), fc, FFN: K-tile pairs are adjacent 128-ch blocks.
# The key-padding mask is folded into v65 (value rows and the ones-column
# multiplied by 0/1), so exp needs no per-chunk bias and one Activation op
# covers two score chunks (a 2-bank PSUM tile).
import numpy as np
import ml_dtypes
from contextlib import ExitStack

import concourse.bass as bass
import concourse.mybir as mybir
import concourse.tile as tile
from concourse import bacc
from concourse.bass_utils import run_bass_kernel_spmd
from concourse.masks import make_identity

BF16 = mybir.dt.bfloat16
F32 = mybir.dt.float32
FP8 = mybir.dt.float8e4
DR = mybir.MatmulPerfMode.DoubleRow
AF = mybir.ActivationFunctionType
ALU = mybir.AluOpType
AX = mybir.AxisListType

B, T, D, H, DK, KW, FF = 4, 1024, 1024, 16, 64, 13, 4096
TQ = 512           # query tokens per core
P = 128
NHW = 768          # noisy halo window rows (zero-padded on host)
NTW = 528          # nt2 plane width (524 used + 4 pad)
CTW = 1040         # ct2 plane width (1036 used + 4 pad)
EPS1, EPS2 = 1e-5, 1e-6
WS = 32.0          # host-side weight scale (power of two)
WSI = 1.0 / WS


def pair_b(ap2d):
    """[p, N] -> [p, 2, N] stride-0 plane broadcast (for double-q scores)."""
    p, n = ap2d.shape
    return ap2d.unsqueeze(1).broadcast_to((p, 2, n))


def build_nc():
    nc = bacc.Bacc("TRN2", target_bir_lowering=False, debug=False,
                   num_devices=8)
    dt = {}

    def din(name, shape, dtype):
        dt[name] = nc.dram_tensor(name, list(shape), dtype,
                                  kind="ExternalInput").ap()

    din("noisyH", (NHW, D), BF16)      # rows [t0-128, t0+640), zero padded
    din("clean", (T, D), BF16)
    din("lnst", (36, P), F32)          # host LN stats: r2[6],b2[6],r1[4],
    #                                     b1[4], rc[8],bc[8]
    din("mod", (10, D), F32)           # sh_m,1+sc_m,g_m,sh_f,1+sc_f,g_f,
    #                                     shL,(1+sc)L,shR,(1+sc)R (edge-masked)
    din("clng", (D,), F32)
    din("clnb", (D,), F32)
    din("wq", (P, 8, 7, 2, P), FP8)    # [cin][pair][tap-pair j][plane][cout]
    din("wk", (P, 8, 7, 2, P), FP8)
    din("wv", (P, 8, 7, 2, P), FP8)
    din("bq", (D,), F32)
    din("bk", (D,), F32)
    din("bv", (D,), F32)
    din("mk", (P, 8, 2, DK), FP8)      # key mask replicated (part,chunk,h2,dk)
    din("fcw", (P, 8, 4, 2, P), FP8)   # [ic][m][j][plane][oc]
    din("fcb", (D,), F32)
    din("w1h", (P, 32, 4, 2, P), FP8)  # [ic][m][j][plane][oc] hi level
    din("w1l", (P, 32, 4, 2, P), FP8)  # lo level (residual)
    din("fb1", (FF,), F32)
    din("w2h", (P, 8, 16, 2, P), FP8)
    din("w2l", (P, 8, 16, 2, P), FP8)
    din("fb2", (D,), F32)
    out_ap = nc.dram_tensor("out", [TQ, D], F32, kind="ExternalOutput").ap()

    with tile.TileContext(nc) as tc:
        _emit(tc, dt, out_ap)
    nc.compile()
    return nc


def _emit(tc, dt, out_ap):
    nc = tc.nc
    with ExitStack() as ctx:
        const = ctx.enter_context(tc.tile_pool(name="const", bufs=1))
        small = ctx.enter_context(tc.tile_pool(name="small", bufs=4))
        lnio = ctx.enter_context(tc.tile_pool(name="lnio", bufs=4))
        big = ctx.enter_context(tc.tile_pool(name="bigsb", bufs=1))
        fstr = ctx.enter_context(tc.tile_pool(name="fstr", bufs=2))
        pcv = ctx.enter_context(tc.tile_pool(name="pcv", bufs=1, space="PSUM"))
        psc = ctx.enter_context(tc.tile_pool(name="psc", bufs=2, space="PSUM"))
        ptp = ctx.enter_context(tc.tile_pool(name="ptp", bufs=2, space="PSUM"))
        ppv = ctx.enter_context(tc.tile_pool(name="ppv", bufs=1, space="PSUM"))

        lnst_s = const.tile([P, 36], F32)
        nc.sync.dma_start(lnst_s, dt["lnst"].rearrange("n p -> p n"))
        ident = const.tile([P, P], BF16)
        make_identity(nc, ident)
        e_1 = const.tile([P, 1], F32)
        nc.vector.memset(e_1, EPS1)
        e_12 = const.tile([P, 1], F32)
        nc.vector.memset(e_12, EPS1 * EPS2)
        e_2 = const.tile([P, 1], F32)
        nc.vector.memset(e_2, EPS2)
        EPS_T = {"1": e_1, "12": e_12, "2": e_2}

        def stats(x, eng_sq):
            """(mu, var) [p,1] f32 for rows of x [p, D]."""
            s = small.tile([P, 1], F32, tag="st_s")
            nc.vector.reduce_sum(s, x, axis=AX.X)
            sq = small.tile([P, 1], F32, tag="st_sq")
            scr = small.tile([P, D], BF16, tag="st_scr", bufs=2)
            nc.scalar.activation(scr, x, AF.Square, accum_out=sq)
            mu = small.tile([P, 1], F32, tag="st_mu")
            nc.vector.tensor_scalar_mul(mu, s, 1.0 / D)
            msq = small.tile([P, 1], F32, tag="st_msq")
            nc.vector.tensor_tensor(msq, mu, mu, ALU.mult)
            var = small.tile([P, 1], F32, tag="st_var")
            nc.vector.tensor_scalar(var, sq, 1.0 / D, msq,
                                    ALU.mult, ALU.subtract)
            return mu, var

        def rs_beta(mu, var, eps_key, scale, tag):
            """(r, beta) with r = 1/sqrt(var*scale+eps), beta = -mu*r."""
            st = small.tile([P, 1], F32, tag=tag + "_s")
            nc.scalar.activation(st, var, AF.Sqrt, bias=EPS_T[eps_key],
                                 scale=scale)
            rt = small.tile([P, 1], F32, tag=tag + "_r")
            nc.vector.reciprocal(rt, st)
            bt = small.tile([P, 1], F32, tag=tag + "_b")
            nc.vector.tensor_tensor(bt, mu, rt, ALU.mult)
            nc.vector.tensor_scalar_mul(bt, bt, -1.0)
            return rt, bt

        xres = big.tile([P, 4, D], F32)        # y1 rows [t0, t0+512); then x
        abc_cm = tc.tile_pool(name="abc", bufs=1)
        abc = abc_cm.__enter__()
        nt2 = [abc.tile([P, 2, NTW], FP8, name=f"nt2_{m}") for m in range(8)]
        ct2 = [abc.tile([P, 2, CTW], FP8, name=f"ct2_{m}") for m in range(8)]
        qT = [abc.tile([P, TQ], FP8, name=f"qT_{m}") for m in range(8)]


        # one-time pad zeroing (tiles are persistent)
        for m in range(8):
            nc.gpsimd.memset(nt2[m][:, 0, 524:528], 0.0)
            nc.gpsimd.memset(nt2[m][:, 1, 523:528], 0.0)
            nc.gpsimd.memset(ct2[m][:, 0, 0:6], 0.0)
            nc.gpsimd.memset(ct2[m][:, 0, 1030:1040], 0.0)
            nc.gpsimd.memset(ct2[m][:, 1, 1029:1040], 0.0)

        # ---- Phase A: noisy stats -> y1 (xres) + y2 -> nt2 + Q convs --------
        def chanvec(name, w=8):
            t = const.tile([P, w], F32, tag=f"cv_{name}")
            nc.sync.dma_start(t, dt[name].rearrange("(m p) -> p m", p=P))
            return t

        bq_s, bk_s, bv_s = chanvec("bq"), chanvec("bk"), chanvec("bv")
        fcb_s, fb2_s = chanvec("fcb"), chanvec("fb2")
        clng_s, clnb_s = chanvec("clng"), chanvec("clnb")
        fb1_s = chanvec("fb1", 32)
        mod_s = const.tile([P, 10, 8], F32)
        nc.sync.dma_start(mod_s, dt["mod"].rearrange("s (m p) -> p s m", p=P))
        sh_msa, sc_msa, g_msa = mod_s[:, 0, :], mod_s[:, 1, :], mod_s[:, 2, :]
        sh_mlp, sc_mlp, g_mlp = mod_s[:, 3, :], mod_s[:, 4, :], mod_s[:, 5, :]
        shL, scL = mod_s[:, 6, :], mod_s[:, 7, :]
        shR, scR = mod_s[:, 8, :], mod_s[:, 9, :]
        mk_s = const.tile([P, 8, 2, DK], FP8)
        nc.sync.dma_start(mk_s, dt["mk"])

        with tc.tile_pool(name="lnA", bufs=1) as lnA:
            y2 = [lnA.tile([P, D], BF16, name=f"y2_{r}") for r in range(6)]
            for r in range(6):
                xt = lnio.tile([P, D], BF16, tag="ln_in", bufs=3)
                nc.sync.dma_start(xt, dt["noisyH"][r * P:(r + 1) * P, :])
                nc.vector.tensor_scalar(y2[r], xt, lnst_s[:, r:r + 1],
                                        lnst_s[:, 6 + r:7 + r],
                                        ALU.mult, ALU.add)
                if 1 <= r <= 4:
                    nc.gpsimd.tensor_scalar(xres[:, r - 1, :], xt,
                                            lnst_s[:, 11 + r:12 + r],
                                            lnst_s[:, 15 + r:16 + r],
                                            ALU.mult, ALU.add)
            for m in range(8):
                bA = ptp.tile([P, 4, P], BF16, tag="tpb")
                bB = ptp.tile([P, 4, P], BF16, tag="tpb")
                for r in range(6):
                    dst = bA[:, r, :] if r < 4 else bB[:, r - 4, :]
                    nc.tensor.transpose(dst, y2[r][:, m * P:(m + 1) * P],
                                        ident)
                mm, ms = sc_msa[:, m:m + 1], sh_msa[:, m:m + 1]
                nc.vector.tensor_scalar(nt2[m][:, 0, 0:6], bA[:, 0, 122:128],
                                        scL[:, m:m + 1], shL[:, m:m + 1],
                                        ALU.mult, ALU.add)
                nc.vector.tensor_scalar(
                    nt2[m][:, 0, 6:390],
                    bA.rearrange("p a b -> p (a b)")[:, 128:512],
                    mm, ms, ALU.mult, ALU.add)
                nc.vector.tensor_scalar(nt2[m][:, 0, 390:518],
                                        bB[:, 0, :], mm, ms,
                                        ALU.mult, ALU.add)
                nc.vector.tensor_scalar(nt2[m][:, 0, 518:524], bB[:, 1, 0:6],
                                        scR[:, m:m + 1], shR[:, m:m + 1],
                                        ALU.mult, ALU.add)
                nc.sync.dma_start(nt2[m][:, 1, 0:523], nt2[m][:, 0, 1:524])
        for hp in range(8):
            if hp % 4 == 0:
                wqs = abc.tile([P, 4, 7, 2, P], FP8, tag="wqs", bufs=2)
                nc.sync.dma_start(wqs, dt["wq"][:, hp:hp + 4])
            ps = pcv.tile([P, TQ], F32, tag="conv")
            for j in range(7):
                nc.tensor.matmul(ps, wqs[:, hp % 4, j],
                                 nt2[hp][:, :, 2 * j:2 * j + TQ],
                                 start=(j == 0), stop=(j == 6), perf_mode=DR)
            nc.vector.tensor_scalar(qT[hp], ps, WSI, bq_s[:, hp:hp + 1],
                                    ALU.mult, ALU.add)

        # ---- Phase B: clean stats -> y -> ct2 -------------------------------
        with tc.tile_pool(name="lnB", bufs=1) as lnB:
            yc = [lnB.tile([P, D], BF16, name=f"yc_{r}") for r in range(8)]
            for r in range(8):
                xt = lnio.tile([P, D], BF16, tag="ln_in", bufs=3)
                nc.sync.dma_start(xt, dt["clean"][r * P:(r + 1) * P, :])
                nc.vector.tensor_scalar(yc[r], xt, lnst_s[:, 20 + r:21 + r],
                                        lnst_s[:, 28 + r:29 + r],
                                        ALU.mult, ALU.add)
            for m in range(8):
                gg, bb = clng_s[:, m:m + 1], clnb_s[:, m:m + 1]
                for half in range(2):
                    bk4 = ptp.tile([P, 4, P], BF16, tag="tpb")
                    for r4 in range(4):
                        nc.tensor.transpose(
                            bk4[:, r4, :],
                            yc[half * 4 + r4][:, m * P:(m + 1) * P], ident)
                    nc.vector.tensor_scalar(
                        ct2[m][:, 0, 6 + half * 512:518 + half * 512],
                        bk4.rearrange("p a b -> p (a b)"), gg, bb,
                        ALU.mult, ALU.add)
                nc.sync.dma_start(ct2[m][:, 1, 0:1039], ct2[m][:, 0, 1:1040])

        # ---- Phase C: per-pair K/V conv + attention -------------------------
        attp_cm = tc.tile_pool(name="attp", bufs=1)
        attp = attp_cm.__enter__()
        attnT = attp.tile([P, 8, TQ], FP8)
        attnQ = attp.tile([P, 4, H, DK], BF16)
        with tc.tile_pool(name="hpool", bufs=2) as hpool:
            for hp in range(8):
                if hp % 4 == 0:
                    wks = abc.tile([P, 4, 7, 2, P], FP8, tag="wks", bufs=2)
                    nc.sync.dma_start(wks, dt["wk"][:, hp:hp + 4])
                    wvs = abc.tile([P, 4, 7, 2, P], FP8, tag="wvs", bufs=2)
                    nc.sync.dma_start(wvs, dt["wv"][:, hp:hp + 4])
                kT = hpool.tile([P, 2, TQ], FP8, tag="kT")
                for c in range(2):
                    ps = pcv.tile([P, TQ], F32, tag="conv")
                    for j in range(7):
                        nc.tensor.matmul(
                            ps, wks[:, hp % 4, j],
                            ct2[hp][:, :, 2 * j + c * TQ:2 * j + (c + 1) * TQ],
                            start=(j == 0), stop=(j == 6), perf_mode=DR)
                    nc.vector.tensor_scalar(kT[:, c, :], ps, WSI,
                                            bk_s[:, hp:hp + 1],
                                            ALU.mult, ALU.add)
                vT = hpool.tile([P, 2, TQ], BF16, tag="vT")
                for c in range(2):
                    ps = pcv.tile([P, TQ], F32, tag="conv")
                    for j in range(7):
                        nc.tensor.matmul(
                            ps, wvs[:, hp % 4, j],
                            ct2[hp][:, :, 2 * j + c * TQ:2 * j + (c + 1) * TQ],
                            start=(j == 0), stop=(j == 6), perf_mode=DR)
                    nc.vector.tensor_scalar(vT[:, c, :], ps, WSI,
                                            bv_s[:, hp:hp + 1],
                                            ALU.mult, ALU.add)
                # v65: token-major masked values + masked ones column
                v65 = hpool.tile([P, 8, 2, 80], FP8, tag="v65")
                nc.vector.tensor_copy(v65[:, :, :, 64], mk_s[:, :, :, 0])
                vTf = vT.rearrange("p a b -> p (a b)")
                for half in range(2):
                    bk4 = ptp.tile([P, 4, P], BF16, tag="tpb")
                    for c4 in range(4):
                        cc = half * 4 + c4
                        nc.tensor.transpose(bk4[:, c4, :],
                                            vTf[:, cc * P:(cc + 1) * P],
                                            ident)
                    nc.vector.tensor_tensor(
                        v65[:, half * 4:half * 4 + 4, :, 0:64],
                        bk4.rearrange("p a (h d) -> p a h d", h=2),
                        mk_s[:, half * 4:half * 4 + 4, :, :], ALU.mult)
                kTf = kT.rearrange("p a b -> p (a b)")
                for hh in range(2):
                    h = 2 * hp + hh
                    rows = slice(hh * DK, (hh + 1) * DK)
                    pT = hpool.tile([P, 8, TQ], FP8, tag="pT")
                    for c2 in range(4):
                        ps = psc.tile([P, 2, TQ], F32, tag="sc")
                        for c in range(2):
                            cc = 2 * c2 + c
                            nc.tensor.matmul(
                                ps[:, c, :],
                                pair_b(kTf[rows, cc * P:(cc + 1) * P]),
                                pair_b(qT[hp][rows, :]),
                                start=True, stop=True, perf_mode=DR)
                        nc.scalar.activation(
                            pT[:, 2 * c2:2 * c2 + 2, :].rearrange(
                                "p a b -> p (a b)"),
                            ps.rearrange("p a b -> p (a b)"), AF.Exp,
                            scale=0.0625)
                    pv = ppv.tile([P, 4, 65], F32, tag="pv")
                    for jq in range(4):
                        for c2 in range(4):
                            nc.tensor.matmul(
                                pv[:, jq, :],
                                pT[:, 2 * c2:2 * c2 + 2, jq * P:(jq + 1) * P],
                                v65[:, 2 * c2:2 * c2 + 2, hh, 0:65],
                                start=(c2 == 0), stop=(c2 == 3), perf_mode=DR)
                    rec = small.tile([P, 4], F32, tag="rec")
                    nc.vector.reciprocal(rec, pv[:, :, 64])
                    for jq in range(4):
                        nc.vector.tensor_scalar_mul(attnQ[:, jq, h, :],
                                                    pv[:, jq, 0:64],
                                                    rec[:, jq:jq + 1])

        abc_cm.__exit__(None, None, None)

        # ---- Phase D: attnT transposes + fc + residual ----------------------
        aqf = attnQ.rearrange("p a h d -> p (a h d)")
        for hp in range(8):
            bk4 = ptp.tile([P, 4, P], BF16, tag="tpb")
            for jq in range(4):
                nc.tensor.transpose(
                    bk4[:, jq, :],
                    aqf[:, jq * 1024 + hp * P:jq * 1024 + (hp + 1) * P],
                    ident)
            nc.vector.tensor_copy(attnT[:, hp, :],
                                  bk4.rearrange("p a b -> p (a b)"))
        fcws = attp.tile([P, 8, 4, 2, P], FP8, name="fcws")
        nc.sync.dma_start(fcws, dt["fcw"])
        for m2 in range(4):
            ps2 = psc.tile([P, 2, TQ], F32, tag="sc")
            for half in range(2):
                for j in range(4):
                    nc.tensor.matmul(ps2[:, half, :],
                                     fcws[:, 2 * m2 + half, j],
                                     attnT[:, 2 * j:2 * j + 2, :],
                                     start=(j == 0), stop=(j == 3),
                                     perf_mode=DR)
          # per-m evacs below read ps2 planes
            fcgs2 = []
            for half in range(2):
                m = 2 * m2 + half
                ps = ps2[:, half, :]
                fcg = lnio.tile([P, TQ], BF16, tag="fcg", bufs=4)
                gb = small.tile([P, 1], F32, tag="gb")
                nc.vector.tensor_tensor(gb, fcb_s[:, m:m + 1],
                                        g_msa[:, m:m + 1], ALU.mult)
                gsc = small.tile([P, 1], F32, tag="gsc")
                nc.vector.tensor_scalar_mul(gsc, g_msa[:, m:m + 1], WSI)
                nc.scalar.activation(fcg, ps, AF.Identity, bias=gb, scale=gsc)
                fcgs2.append(fcg)
            for half in range(2):
                m = 2 * m2 + half
                fcg = fcgs2[half]
                bk4 = ptp.tile([P, 4, P], BF16, tag="tpb")
                for jq in range(4):
                    nc.tensor.transpose(bk4[:, jq, :],
                                        fcg[:, jq * P:(jq + 1) * P], ident)
                nc.vector.tensor_tensor(xres[:, :, m * P:(m + 1) * P], bk4,
                                        xres[:, :, m * P:(m + 1) * P],
                                        ALU.add)

        # ---- Phase E: LN3 + mlp modulation -> n2T ---------------------------
        attp_cm.__exit__(None, None, None)
        abc_cm.__exit__(None, None, None)
        n2p_cm = tc.tile_pool(name="n2p", bufs=1)
        n2p = n2p_cm.__enter__()
        n2hi = n2p.tile([P, 8, TQ], FP8)
        n2lo = n2p.tile([P, 8, TQ], FP8)
        with tc.tile_pool(name="lnE", bufs=1) as lnE:
            l3 = [lnE.tile([P, D], BF16, name=f"l3_{s}") for s in range(4)]
            for s in range(4):
                mu, var = stats(xres[:, s, :], "act" if s % 2 else "dve")
                r3, b3 = rs_beta(mu, var, "2", 1.0, "r3")
                nc.vector.tensor_scalar(l3[s], xres[:, s, :], r3, b3,
                                        ALU.mult, ALU.add)
            for m in range(8):
                bk4 = ptp.tile([P, 4, P], BF16, tag="tpb")
                for s in range(4):
                    nc.tensor.transpose(bk4[:, s, :],
                                        l3[s][:, m * P:(m + 1) * P], ident)
                bk4f = bk4.rearrange("p a b -> p (a b)")
                nc.vector.tensor_copy(n2hi[:, m, :], bk4f)
                nc.vector.tensor_tensor(n2lo[:, m, :], bk4f, n2hi[:, m, :],
                                        ALU.subtract)

        # ---- Phase F: FFN ---------------------------------------------------
        with tc.tile_pool(name="ffp", bufs=1) as ffp:
            ffahi = ffp.tile([P, 32, TQ], FP8)
            ffalo = ffp.tile([P, 32, TQ], FP8)
            for m8 in range(4):
                w1ch = fstr.tile([P, 8, 4, 2, P], FP8, tag="w1ch", bufs=2)
                nc.sync.dma_start(w1ch, dt["w1h"][:, m8 * 8:(m8 + 1) * 8])
                w1cl = fstr.tile([P, 8, 4, 2, P], FP8, tag="w1cl", bufs=2)
                nc.sync.dma_start(w1cl, dt["w1l"][:, m8 * 8:(m8 + 1) * 8])
                for m2 in range(4):
                    ps2 = psc.tile([P, 2, TQ], F32, tag="sc")
                    for half in range(2):
                        m_ = 2 * m2 + half
                        m = m8 * 8 + m_
                        first = True
                        for j in range(4):
                            xh = n2hi[:, 2 * j:2 * j + 2, :]
                            xl = n2lo[:, 2 * j:2 * j + 2, :]
                            for wgt, rhs in ((w1ch, xh), (w1ch, xl),
                                             (w1cl, xh)):
                                nc.tensor.matmul(
                                    ps2[:, half, :], wgt[:, m_, j], rhs,
                                    start=first,
                                    stop=(j == 3 and rhs is xh
                                          and wgt is w1cl),
                                    perf_mode=DR)
                                first = False
                        ffabf = lnio.tile([P, TQ], BF16, tag="ffabf",
                                          bufs=3)
                        nc.scalar.activation(ffabf, ps2[:, half, :],
                                             AF.Gelu_apprx_tanh,
                                             bias=fb1_s[:, m:m + 1],
                                             scale=WSI)
                        nc.gpsimd.tensor_copy(ffahi[:, m, :], ffabf)
                        nc.vector.tensor_tensor(ffalo[:, m, :], ffabf,
                                                ffahi[:, m, :], ALU.subtract)
            for m2 in range(4):
                ps2 = psc.tile([P, 2, TQ], F32, tag="sc")
                ffgs2 = []
                for half in range(2):
                    m = 2 * m2 + half
                    w2ch = fstr.tile([P, 16, 2, P], FP8, tag="w2ch", bufs=3)
                    nc.sync.dma_start(w2ch, dt["w2h"][:, m])
                    w2cl = fstr.tile([P, 16, 2, P], FP8, tag="w2cl", bufs=3)
                    nc.sync.dma_start(w2cl, dt["w2l"][:, m])
                    first = True
                    for j in range(16):
                        xh = ffahi[:, 2 * j:2 * j + 2, :]
                        xl = ffalo[:, 2 * j:2 * j + 2, :]
                        for wgt, rhs in ((w2ch, xh), (w2ch, xl), (w2cl, xh)):
                            nc.tensor.matmul(
                                ps2[:, half, :], wgt[:, j], rhs,
                                start=first,
                                stop=(j == 15 and rhs is xh and wgt is w2cl),
                                perf_mode=DR)
                            first = False
                    ps = ps2[:, half, :]
                    ffg = lnio.tile([P, TQ], BF16, tag="ffg", bufs=4)
                    gb = small.tile([P, 1], F32, tag="gb2")
                    nc.vector.tensor_tensor(gb, fb2_s[:, m:m + 1],
                                            g_mlp[:, m:m + 1], ALU.mult)
                    gs2 = small.tile([P, 1], F32, tag="gs2")
                    nc.vector.tensor_scalar_mul(gs2, g_mlp[:, m:m + 1], WSI)
                    nc.scalar.activation(ffg, ps, AF.Identity, bias=gb,
                                         scale=gs2)
                    ffgs2.append(ffg)
                for half in range(2):
                    m = 2 * m2 + half
                    ffg = ffgs2[half]
                    bk4 = ptp.tile([P, 4, P], BF16, tag="tpb")
                    for jq in range(4):
                        nc.tensor.transpose(bk4[:, jq, :],
                                            ffg[:, jq * P:(jq + 1) * P],
                                            ident)
                    nc.vector.tensor_tensor(xres[:, :, m * P:(m + 1) * P],
                                            bk4,
                                            xres[:, :, m * P:(m + 1) * P],
                                            ALU.add)

        n2p_cm.__exit__(None, None, None)
        for s in range(4):
            nc.sync.dma_start(out_ap[s * P:(s + 1) * P, :], xres[:, s, :])


# --------------------------- host side --------------------------------------
_NC_CACHE = None
_LAST_INMAPS = None


def _prep_conv_w(w):
    """(D, DK, KW) grouped conv -> [128, 8, 7, 2, 128] fp8 block-diag."""
    f8 = ml_dtypes.float8_e4m3
    wr = w.reshape(H, DK, DK, KW)            # [h, out, cin, tap]
    arr = np.zeros((P, 8, 7, 2, P), np.float32)
    for hp in range(8):
        for g in range(2):
            h = 2 * hp + g
            rs = slice(g * DK, (g + 1) * DK)
            for j in range(7):
                arr[rs, hp, j, 0, rs] = wr[h, :, :, 2 * j].T * WS
                if 2 * j + 1 < KW:
                    arr[rs, hp, j, 1, rs] = wr[h, :, :, 2 * j + 1].T * WS
    return arr.astype(f8)


def _prep_mm_w(wT, nm, nj):
    """wT [K, M] contraction-major -> [128, nm, nj, 2, 128] fp8."""
    f8 = ml_dtypes.float8_e4m3
    arr = wT.reshape(nj, 2, P, nm, P).transpose(2, 3, 0, 1, 4) * WS
    return np.ascontiguousarray(arr).astype(f8)


def kernel(**inputs):
    global _NC_CACHE, _LAST_INMAPS
    if _NC_CACHE is None:
        _NC_CACHE = build_nc()
    nc = _NC_CACHE

    f32 = np.float32
    f8 = ml_dtypes.float8_e4m3
    noisy = np.asarray(inputs["noisy_feats"], f32)
    clean = np.asarray(inputs["clean_feats"], f32)
    t = np.asarray(inputs["t"], f32)
    clean_len = np.asarray(inputs["clean_lengths"]).astype(np.int64)

    assert np.all(np.asarray(inputs["ln_noisy_g"], f32) == 1.0)
    assert np.all(np.asarray(inputs["ln_noisy_b"], f32) == 0.0)

    # AdaLayerNormZero on host (0.02% of FLOPs)
    st = t * (1.0 / (1.0 + np.exp(-t, dtype=f32)))
    emb = st @ np.asarray(inputs["ada_w"], f32).T + \
        np.asarray(inputs["ada_b"], f32)
    sh_msa, sc_msa, g_msa, sh_mlp, sc_mlp, g_mlp = np.split(emb, 6, axis=1)

    wql = _prep_conv_w(np.asarray(inputs["wq"], f32))
    wkl = _prep_conv_w(np.asarray(inputs["wk"], f32))
    wvl = _prep_conv_w(np.asarray(inputs["wv"], f32))
    fcw = _prep_mm_w(np.asarray(inputs["fc_w"], f32).T, 8, 4)

    def hilo(wT, nm, nj):
        ws = wT * WS
        hi = ws.astype(f8).astype(f32)
        lo = ws - hi
        return (_prep_mm_w(hi / WS, nm, nj), _prep_mm_w(lo / WS, nm, nj))

    w2h, w2l = hilo(np.asarray(inputs["ff_w2"], f32).T, 8, 16)
    ff_w1 = np.asarray(inputs["ff_w1"], f32)
    fb1_base = np.asarray(inputs["ff_b1"], f32)

    common = dict(
        clng=np.asarray(inputs["ln_clean_g"], f32).copy(),
        clnb=np.asarray(inputs["ln_clean_b"], f32).copy(),
        wq=wql, wk=wkl, wv=wvl,
        bq=np.asarray(inputs["bq"], f32).copy(),
        bk=np.asarray(inputs["bk"], f32).copy(),
        bv=np.asarray(inputs["bv"], f32).copy(),
        fcw=fcw, fcb=np.asarray(inputs["fc_b"], f32).copy(),
        w2h=w2h, w2l=w2l, fb2=np.asarray(inputs["ff_b2"], f32).copy(),
    )

    bfd = ml_dtypes.bfloat16
    in_maps = []
    for i in range(8):
        b, half = i // 2, i % 2
        t0 = half * TQ
        noisyH = np.zeros((NHW, D), f32)
        lo, hi = t0 - P, t0 + 640
        clo, chi = max(lo, 0), min(hi, T)
        noisyH[clo - lo:chi - lo] = noisy[b, clo:chi]
        # host-side LN stats (exact f32); applies happen on-chip
        lnst = np.zeros((36, P), f32)
        nmu = noisyH.mean(1)
        nvar = noisyH.var(1)
        r2v = 1.0 / np.sqrt(nvar * (1 + EPS2) + EPS1 * EPS2)
        r1v = 1.0 / np.sqrt(nvar + EPS1)
        lnst[0:6] = r2v.reshape(6, P)
        lnst[6:12] = (-nmu * r2v).reshape(6, P)
        lnst[12:16] = r1v.reshape(6, P)[1:5]
        lnst[16:20] = (-nmu * r1v).reshape(6, P)[1:5]
        cmu = clean[b].mean(1)
        cvar = clean[b].var(1)
        rcv = 1.0 / np.sqrt(cvar + EPS1)
        lnst[20:28] = rcv.reshape(8, P)
        lnst[28:36] = (-cmu * rcv).reshape(8, P)
        mvec = (np.arange(T) < clean_len[b]).astype(f32)
        mk = np.broadcast_to(
            mvec.reshape(8, P).T[:, :, None, None], (P, 8, 2, DK)).astype(f8)
        eL = 0.0 if half == 0 else 1.0
        eR = 1.0 if half == 0 else 0.0
        mod = np.stack([
            sh_msa[b], 1.0 + sc_msa[b], g_msa[b],
            sh_mlp[b], 1.0 + sc_mlp[b], g_mlp[b],
            sh_msa[b] * eL, (1.0 + sc_msa[b]) * eL,
            sh_msa[b] * eR, (1.0 + sc_msa[b]) * eR,
        ]).astype(f32)
        w1b = ff_w1 * (1.0 + sc_mlp[b])[None, :]
        w1h, w1l = hilo(w1b.T, 32, 4)
        fb1 = fb1_base + ff_w1 @ sh_mlp[b]
        m = dict(common)
        m.update(noisyH=noisyH.astype(bfd), clean=clean[b].astype(bfd),
                 mod=mod, mk=np.ascontiguousarray(mk), lnst=lnst,
                 w1h=w1h, w1l=w1l, fb1=fb1)
        in_maps.append(m)

    _LAST_INMAPS = in_maps
    res = run_bass_kernel_spmd(nc, in_maps, core_ids=list(range(8)))
    out = np.empty((B, T, D), f32)
    for i in range(8):
        b, half = i // 2, i % 2
        out[b, half * TQ:(half + 1) * TQ] = res.results[i]["out"]
    return out


if __name__ == "__main__":
    build_nc()
    print("build ok")
